# revision 1
# baseline (speedup 1.0000x reference)
"""Trainium2 Bass kernel: CrossAttentionFusion (dense transformer block pair).

Math notes (vs the reference):
  - seq_len-1 cross attention: softmax over a single key == 1, so
    mha1(q_in, kv_in) == kv_in @ (Wo@Wv).T + (Wo@bv + bo).  q/k projections are
    dead code; the two projections fuse into ONE 768x768 matmul (host-fused).
  - Transposed layout: activations live as [feature, batch]; matmuls are
    lhsT(=W.T, stationary) x rhs(=x.T, moving) -> out = (x@W.T).T.
    LayerNorm reduces over features (= partitions) with a ones-vector matmul on
    the PE; per-sample stats are broadcast back over partitions with K=1 ones
    matmuls.
  - Data-parallel over batch: 16384 rows -> 8 cores x 2048.
  - bf16 matmul operands (2x PE throughput vs f32), f32 PSUM accumulation.
  - Software pipeline with 1-strip skew; LN broadcast+apply for strip s-1 is
    emitted mid-attention of strip s so the PE never waits on the LN row-stat
    chain (which would also re-throttle the PE clock via HAM).
"""

import numpy as np
import ml_dtypes

import concourse.bass as bass
from concourse import bacc, tile, mybir
from concourse.bass_utils import run_bass_kernel_spmd

BF16 = ml_dtypes.bfloat16
DT_BF = mybir.dt.bfloat16
DT_F32 = mybir.dt.float32
AF = mybir.ActivationFunctionType
ALU = mybir.AluOpType

B_FULL, E, H = 16384, 768, 8
F = 4 * E  # 3072
N_CORES = 8
BS = B_FULL // N_CORES  # 2048
EPS = 1e-5
P = 128
KE = E // P  # 6
KF = F // P  # 24


def build(bs=BS, strip=512, sub=512, use_gelu=True, num_devices=N_CORES,
          sim_safe=False):
    """Emit the per-core Bass program (SPMD: same program on every core)."""
    nstrip = bs // strip
    nsub = strip // sub
    assert nstrip * strip == bs and nsub * sub == strip

    nc = bacc.Bacc(
        "TRN2", target_bir_lowering=False, debug=False, num_devices=num_devices
    )

    # ---- DRAM I/O ----
    d_img = nc.dram_tensor("imageT", [E, bs], DT_BF, kind="ExternalInput")
    d_txt = nc.dram_tensor("textT", [E, bs], DT_BF, kind="ExternalInput")
    d_watt = {
        "it": nc.dram_tensor("watt_it", [E, E], DT_BF, kind="ExternalInput"),
        "ti": nc.dram_tensor("watt_ti", [E, E], DT_BF, kind="ExternalInput"),
    }
    d_wfp = nc.dram_tensor("wfp", [2 * E, E], DT_BF, kind="ExternalInput")
    d_w1 = {
        p: nc.dram_tensor(f"w1_{p}", [E, F], DT_BF, kind="ExternalInput")
        for p in ("fi", "ft")
    }
    d_w2 = {
        p: nc.dram_tensor(f"w2_{p}", [F, E], DT_BF, kind="ExternalInput")
        for p in ("fi", "ft")
    }
    bias_specs = {
        "batt_it": KE, "g_img": KE, "b_img": KE, "b1_fi": KF, "b2_fi": KE,
        "batt_ti": KE, "g_txt": KE, "b_txt": KE, "b1_ft": KF, "b2_ft": KE,
        "bfp": KE, "g_fp": KE, "b_fp_ln": KE,
    }
    d_bias = {
        n: nc.dram_tensor(n, [P, k], DT_F32, kind="ExternalInput")
        for n, k in bias_specs.items()
    }
    d_out = nc.dram_tensor("outT", [E, bs], DT_F32, kind="ExternalOutput")

    def dview(d):  # [E|2E, bs] dram -> [p, kt, n] view
        return d.ap().rearrange("(kt p) n -> p kt n", p=P)

    with tile.TileContext(nc) as tc:
        from contextlib import ExitStack

        with ExitStack() as ctx:
            const = ctx.enter_context(tc.tile_pool(name="const", bufs=1))
            pin = ctx.enter_context(tc.tile_pool(name="pin", bufs=2))
            pwork = ctx.enter_context(tc.tile_pool(name="pwork", bufs=2))
            ph = ctx.enter_context(tc.tile_pool(name="ph", bufs=1))
            prow = ctx.enter_context(tc.tile_pool(name="prow", bufs=1))
            pst = ctx.enter_context(tc.tile_pool(name="pst", bufs=3))
            pps = ctx.enter_context(
                tc.tile_pool(name="pps", bufs=2, space=bass.MemorySpace.PSUM)
            )
            pdram = ctx.enter_context(
                tc.tile_pool(name="pdram", bufs=1, space=bass.MemorySpace.DRAM)
            )

            # ---- constants needed for SP1 start (small, DMA'd first) ----
            ones_sb = const.tile([P, P], DT_BF)
            nc.vector.memset(ones_sb[:], 1.0)
            eps_sb = const.tile([1, 1], DT_F32)
            nc.vector.memset(eps_sb[:], EPS)
            watt_sb = {
                pfx: const.tile(
                    [P, KE, E], DT_BF, tag=f"watt_{pfx}", name=f"watt_{pfx}"
                )
                for pfx in ("it", "ti")
            }
            for k in range(KE):
                nc.sync.dma_start(
                    watt_sb["it"][:, k, :], dview(d_watt["it"])[:, k, :]
                )
            bias_sb = {}
            for n, k in bias_specs.items():
                t = const.tile([P, k], DT_F32, tag=f"bias_{n}")
                nc.sync.dma_start(t[:], d_bias[n].ap())
                bias_sb[n] = t
            wfp_sb = const.tile([P, 2 * KE, E], DT_BF)
            # (watt_ti / wfp DMAs are emitted at SP2/SP3 start, see below)

            # ---- internal DRAM trunk: per-strip tiles for fine-grained deps --
            d_img2 = [
                pdram.tile([P, KE, strip], DT_BF, tag=f"img2_{s}", name=f"img2_{s}")
                for s in range(nstrip)
            ]
            d_txt2 = [
                pdram.tile([P, KE, strip], DT_BF, tag=f"txt2_{s}", name=f"txt2_{s}")
                for s in range(nstrip)
            ]

            # ---------- helpers ----------
            def load_strip_ext(dsrc, sl, tag):
                t = pin.tile([P, KE, strip], DT_BF, tag=tag, name=f"in_{tag}")
                nc.sync.dma_start(t[:], dsrc[:, :, sl])
                return t

            def load_strip_trunk(dtile, tag):
                t = pin.tile([P, KE, strip], DT_BF, tag=tag, name=f"in_{tag}")
                nc.sync.dma_start(t[:], dtile[:])
                return t

            def dense_att(rhs_t, resid_t, w_sb, b_sb, mid_hook=None):
                """r[m] = (x @ Wc.T).T[m] + b[m] + resid[m]  (bf16 out)."""
                r = pwork.tile([P, KE, strip], DT_BF, tag="r1", name="r1")
                for m in range(KE):
                    ps = pps.tile([P, strip], DT_F32, tag="mm", bufs=4, name="ps")
                    for k in range(KE):
                        nc.tensor.matmul(
                            ps[:],
                            w_sb[:, k, m * P:(m + 1) * P],
                            rhs_t[:, k, :],
                            start=(k == 0),
                            stop=(k == KE - 1),
                        )
                    nc.vector.scalar_tensor_tensor(
                        r[:, m, :], ps[:], b_sb[:, m:m + 1], resid_t[:, m, :],
                        ALU.add, ALU.add,
                    )
                    if m == 3 and mid_hook is not None:
                        mid_hook()
                return r

            def ln_presum(r):
                """DVE feature pre-sums of r and r^2 -> [P,strip] bf16 pair."""
                s = pwork.tile([P, strip], DT_BF, tag="s", name="s")
                nc.vector.tensor_tensor(s[:], r[:, 0, :], r[:, 1, :], ALU.add)
                for k in range(2, KE):
                    nc.vector.tensor_tensor(s[:], s[:], r[:, k, :], ALU.add)
                sq = pwork.tile([P, strip], DT_BF, tag="sq", name="sq")
                tmp = pwork.tile([P, strip], DT_BF, tag="sqtmp", name="sqtmp")
                nc.vector.tensor_tensor(sq[:], r[:, 0, :], r[:, 0, :], ALU.mult)
                for k in range(1, KE):
                    nc.vector.tensor_tensor(tmp[:], r[:, k, :], r[:, k, :], ALU.mult)
                    nc.vector.tensor_tensor(sq[:], sq[:], tmp[:], ALU.add)
                return s, sq

            def ln_redrows(ssq):
                """PE partition-reduce + row-stat chain -> (mean, rstd) rows."""
                s, sq = ssq
                red0 = pps.tile([1, strip], DT_F32, tag="hps", bufs=2, name="red0")
                red1 = pps.tile([1, strip], DT_F32, tag="ops", bufs=2, name="red1")
                nc.tensor.matmul(red0[:], ones_sb[:, 0:1], s[:], start=True, stop=True)
                nc.tensor.matmul(red1[:], ones_sb[:, 0:1], sq[:], start=True, stop=True)
                mean_bf = prow.tile([1, strip], DT_BF, tag="mean", name="mean")
                nc.scalar.activation(mean_bf[:], red0[:], AF.Copy, scale=1.0 / E)
                msq = prow.tile([1, strip], DT_F32, tag="msq", name="msq")
                nc.vector.tensor_tensor(msq[:], mean_bf[:], mean_bf[:], ALU.mult)
                var = prow.tile([1, strip], DT_F32, tag="var", name="var")
                nc.vector.scalar_tensor_tensor(
                    var[:], red1[:], 1.0 / E, msq[:], ALU.mult, ALU.subtract
                )
                rstd_bf = prow.tile([1, strip], DT_BF, tag="rstdbf", name="rstdbf")
                if sim_safe:
                    std = prow.tile([1, strip], DT_F32, tag="std", name="std")
                    nc.scalar.activation(std[:], var[:], AF.Sqrt, bias=eps_sb[0:1, 0:1])
                    rstd = prow.tile([1, strip], DT_F32, tag="rstd", name="rstd")
                    nc.vector.reciprocal(rstd[:], std[:])
                    nc.vector.tensor_copy(rstd_bf[:], rstd[:])
                else:
                    nc.scalar.activation(
                        rstd_bf[:], var[:], AF.Abs_reciprocal_sqrt,
                        bias=eps_sb[0:1, 0:1],
                    )
                return mean_bf, rstd_bf

            def ln_bcast_apply(r, rows, out_emit):
                """PE K=1 broadcast of stats over partitions + DVE/ACT apply."""
                mean_bf, rstd_bf = rows
                mb = pps.tile([P, strip], DT_F32, tag="hps", bufs=2, name="mb")
                nc.tensor.matmul(mb[:], ones_sb[0:1, :], mean_bf[:], start=True, stop=True)
                rb = pps.tile([P, strip], DT_F32, tag="ops", bufs=2, name="rb")
                nc.tensor.matmul(rb[:], ones_sb[0:1, :], rstd_bf[:], start=True, stop=True)
                for k in range(KE):
                    t = pwork.tile([P, strip], DT_BF, tag="lnt", name="lnt")
                    nc.vector.tensor_tensor(t[:], r[:, k, :], mb[:], ALU.subtract)
                    nc.vector.tensor_tensor(t[:], t[:], rb[:], ALU.mult)
                    out_emit(k, t)

            def ln_to_x(r, rows, g_sb, b_sb):
                x = [
                    pwork.tile([P, strip], DT_BF, tag=f"xk{k}", name=f"x{k}")
                    for k in range(KE)
                ]

                def emit(k, t):
                    nc.scalar.activation(
                        x[k][:], t[:], AF.Identity,
                        bias=b_sb[:, k:k + 1], scale=g_sb[:, k:k + 1],
                    )

                ln_bcast_apply(r, rows, emit)
                return x

            def ffn(x, w1, w2, b1_sb, b2_sb, dtile):
                """dtile[:, m, :] = x + (gelu(x@W1.T+b1))@W2.T + b2."""
                for si in range(nsub):
                    ssl = slice(si * sub, (si + 1) * sub)
                    h = ph.tile([P, KF, sub], DT_BF, tag="h", name="h")
                    for m in range(KF):
                        hps = pps.tile([P, sub], DT_F32, tag="hps", bufs=2, name="hps")
                        for k in range(KE):
                            nc.tensor.matmul(
                                hps[:], w1[k][:, m * P:(m + 1) * P], x[k][:, ssl],
                                start=(k == 0), stop=(k == KE - 1),
                            )
                        nc.scalar.activation(
                            h[:, m, :], hps[:],
                            AF.Gelu if use_gelu else AF.Identity,
                            bias=b1_sb[:, m:m + 1],
                        )
                    for m in range(KE):
                        ops = pps.tile([P, sub], DT_F32, tag="ops", bufs=2, name="ops")
                        for k in range(KF):
                            nc.tensor.matmul(
                                ops[:], w2[k][:, m * P:(m + 1) * P], h[:, k, :],
                                start=(k == 0), stop=(k == KF - 1),
                            )
                        st = pst.tile([P, sub], DT_BF, tag="stg", name="stg")
                        nc.vector.scalar_tensor_tensor(
                            st[:], ops[:], b2_sb[:, m:m + 1], x[m][:, ssl],
                            ALU.add, ALU.add,
                        )
                        nc.sync.dma_start(dtile[:, m, ssl], st[:])

            def superphase(pfx, rhs_src, res_view, watt, b_att, g_ln, b_ln,
                           w1d, w2d, b1, b2, dout):
                """rhs_src: callable s -> rhs strip tile; res_view: dram view."""
                with tc.tile_pool(name=f"wffn_{pfx}", bufs=1) as wp:
                    w1 = [wp.tile([P, F], DT_BF, tag=f"w1_{k}", name=f"w1{k}")
                          for k in range(KE)]
                    w2 = [wp.tile([P, E], DT_BF, tag=f"w2_{k}", name=f"w2{k}")
                          for k in range(KF)]

                    def load_w():
                        for k in range(KE):
                            nc.sync.dma_start(w1[k][:], w1d.ap()[k * P:(k + 1) * P, :])
                        for k in range(KF):
                            nc.sync.dma_start(w2[k][:], w2d.ap()[k * P:(k + 1) * P, :])

                    pend = None  # (r, rows, strip idx) awaiting bcast/apply+ffn
                    xcur = [None]
                    for s in range(nstrip):
                        sl = slice(s * strip, (s + 1) * strip)
                        rhs_t = rhs_src(s)
                        res_t = load_strip_ext(res_view, sl, "res_in")

                        hook = None
                        if pend is not None:
                            rp, rowsp, _ = pend

                            def hook(rp=rp, rowsp=rowsp):
                                xcur[0] = ln_to_x(rp, rowsp, g_ln, b_ln)

                        r = dense_att(rhs_t, res_t, watt, b_att, mid_hook=hook)
                        ssq = ln_presum(r)
                        if s == 0 and pfx == "fi":
                            load_w()  # after the first strip's work is queued
                            for k in range(KE):
                                nc.sync.dma_start(
                                    watt_sb["ti"][:, k, :],
                                    dview(d_watt["ti"])[:, k, :],
                                )
                        if pend is not None:
                            _, _, sp = pend
                            ffn(xcur[0], w1, w2, b1, b2, dout[sp])
                        if s == 0 and pfx != "fi":
                            load_w()
                            for k in range(2 * KE):
                                nc.sync.dma_start(
                                    wfp_sb[:, k, :], dview(d_wfp)[:, k, :]
                                )
                        rows = ln_redrows(ssq)
                        pend = (r, rows, s)
                    rp, rowsp, sp = pend
                    x = ln_to_x(rp, rowsp, g_ln, b_ln)
                    ffn(x, w1, w2, b1, b2, dout[sp])

            # ---- SP1: image branch (kv = text, residual = image) ----
            superphase(
                "fi",
                lambda s: load_strip_ext(
                    dview(d_txt), slice(s * strip, (s + 1) * strip), "rhs_in"
                ),
                dview(d_img), watt_sb["it"],
                bias_sb["batt_it"], bias_sb["g_img"], bias_sb["b_img"],
                d_w1["fi"], d_w2["fi"], bias_sb["b1_fi"], bias_sb["b2_fi"],
                d_img2,
            )
            # ---- SP2: text branch (kv = img2, residual = text) ----
            superphase(
                "ft",
                lambda s: load_strip_trunk(d_img2[s], "rhs_in"),
                dview(d_txt), watt_sb["ti"],
                bias_sb["batt_ti"], bias_sb["g_txt"], bias_sb["b_txt"],
                d_w1["ft"], d_w2["ft"], bias_sb["b1_ft"], bias_sb["b2_ft"],
                d_txt2,
            )

            # ---- SP3: fused projection + LN + gelu ----
            with tc.tile_pool(name="sp3", bufs=3) as p3:
                outv = dview(d_out)

                def fp_finish(rp3, slp, rowsp):
                    def emit_out(k, t):
                        o = p3.tile([P, strip], DT_F32, tag="of32", name="of32")
                        nc.scalar.activation(
                            o[:], t[:],
                            AF.Gelu if use_gelu else AF.Identity,
                            bias=bias_sb["b_fp_ln"][:, k:k + 1],
                            scale=bias_sb["g_fp"][:, k:k + 1],
                        )
                        nc.sync.dma_start(outv[:, k, slp], o[:])

                    ln_bcast_apply(rp3, rowsp, emit_out)

                # 1-strip skew with in-loop hooks: red(s-1) after m1,
                # finish(s-1) after m5 -- PE never waits on the LN chain.
                stages = []  # per strip dict: r3, sl, ssq, rows
                for s in range(nstrip):
                    sl = slice(s * strip, (s + 1) * strip)
                    a_in = load_strip_trunk(d_img2[s], "rhs_in")
                    b_in = load_strip_trunk(d_txt2[s], "res_in")
                    r3 = pwork.tile([P, KE, strip], DT_BF, tag="r1", name="r3")
                    for m in range(KE):
                        zps = pps.tile([P, strip], DT_F32, tag="mm", bufs=4, name="zps")
                        for k in range(2 * KE):
                            src = a_in if k < KE else b_in
                            nc.tensor.matmul(
                                zps[:], wfp_sb[:, k, m * P:(m + 1) * P],
                                src[:, k % KE, :],
                                start=(k == 0), stop=(k == 2 * KE - 1),
                            )
                        nc.scalar.activation(
                            r3[:, m, :], zps[:], AF.Identity,
                            bias=bias_sb["bfp"][:, m:m + 1],
                        )
                        if m == 1 and stages and "rows" not in stages[-1]:
                            stages[-1]["rows"] = ln_redrows(stages[-1]["ssq"])
                        if m == 5 and stages and not stages[-1].get("done"):
                            st1 = stages[-1]
                            fp_finish(st1["r3"], st1["sl"], st1["rows"])
                            st1["done"] = True
                    stages.append({"r3": r3, "sl": sl, "ssq": ln_presum(r3)})
                last = stages[-1]
                last["rows"] = ln_redrows(last["ssq"])
                fp_finish(last["r3"], last["sl"], last["rows"])

    nc.compile()
    return nc


# ---------------- host side ----------------

_BUILT = {}


def _get_nc(key):
    if key not in _BUILT:
        _BUILT[key] = build(*key)
    return _BUILT[key]


def _packv(v, ktiles):
    return np.ascontiguousarray(np.asarray(v, np.float32).reshape(ktiles, P).T)


def prep_inputs(inputs, bs=BS, n_cores=N_CORES):
    f32 = np.float32
    g = lambda n: np.asarray(inputs[n], f32)
    common = {}
    for pfx in ("it", "ti"):
        wc = g(f"{pfx}_Wo") @ g(f"{pfx}_Wv")
        bc = g(f"{pfx}_Wo") @ g(f"{pfx}_bv") + g(f"{pfx}_bo")
        common[f"watt_{pfx}"] = np.ascontiguousarray(wc.T).astype(BF16)
        common[f"batt_{pfx}"] = _packv(bc, KE)
    common["w1_fi"] = np.ascontiguousarray(g("fi_W1").T).astype(BF16)
    common["w2_fi"] = np.ascontiguousarray(g("fi_W2").T).astype(BF16)
    common["w1_ft"] = np.ascontiguousarray(g("ft_W1").T).astype(BF16)
    common["w2_ft"] = np.ascontiguousarray(g("ft_W2").T).astype(BF16)
    common["wfp"] = np.ascontiguousarray(g("fp_W").T).astype(BF16)
    common["b1_fi"] = _packv(g("fi_b1"), KF)
    common["b2_fi"] = _packv(g("fi_b2"), KE)
    common["b1_ft"] = _packv(g("ft_b1"), KF)
    common["b2_ft"] = _packv(g("ft_b2"), KE)
    common["bfp"] = _packv(g("fp_b"), KE)
    common["g_img"] = _packv(g("ln_img_g"), KE)
    common["b_img"] = _packv(g("ln_img_b"), KE)
    common["g_txt"] = _packv(g("ln_text_g"), KE)
    common["b_txt"] = _packv(g("ln_text_b"), KE)
    common["g_fp"] = _packv(g("fp_ln_g"), KE)
    common["b_fp_ln"] = _packv(g("fp_ln_b"), KE)

    imgT = g("image_embed").T.astype(BF16)
    txtT = g("text_embed").T.astype(BF16)
    in_maps = []
    for c in range(n_cores):
        sl = slice(c * bs, (c + 1) * bs)
        m = dict(common)
        m["imageT"] = np.ascontiguousarray(imgT[:, sl])
        m["textT"] = np.ascontiguousarray(txtT[:, sl])
        in_maps.append(m)
    return in_maps


CFG = (BS, 512, 512, True, N_CORES)


def kernel(**inputs):
    nc = _get_nc(CFG)
    in_maps = prep_inputs(inputs)
    res = run_bass_kernel_spmd(nc, in_maps, core_ids=list(range(N_CORES)))
    out = np.concatenate(
        [res.results[c]["outT"] for c in range(N_CORES)], axis=1
    )  # [E, B]
    return np.ascontiguousarray(out.T).astype(np.float32)



# revision 2
# speedup vs baseline: 1.3811x; 1.3811x over previous
"""Trainium2 Bass kernel: CrossAttentionFusion (dense transformer block pair).

Math notes (vs the reference):
  - seq_len-1 cross attention: softmax over a single key == 1, so
    mha1(q_in, kv_in) == kv_in @ (Wo@Wv).T + (Wo@bv + bo).  q/k projections are
    dead code; the two projections fuse into ONE 768x768 matmul (host-fused).
  - Transposed layout: activations live as [feature, batch]; matmuls are
    lhsT(=W.T, stationary) x rhs(=x.T, moving) -> out = (x@W.T).T.
    LayerNorm reduces over features (= partitions) with a ones-vector matmul on
    the PE; per-sample stats are broadcast back over partitions with K=1 ones
    matmuls.
  - FFN matmuls run in fp8e4 (e4m3) with MatmulPerfMode.DoubleRow: one PE
    instruction contracts TWO 128-row k-subtiles (2x bf16 FLOP rate).  Weights
    are host-quantized with a 256x scale (so sigma~0.02 values sit in e4m3's
    normal range); activations are quantized on the DVE (t8 = 16*t) and by the
    gelu ACT op writing fp8 directly (h8).  Scales unwind in the ACT/DVE
    epilogues (gelu scale = 1/(16*256); FFN2 out scale = 1/256).
  - LayerNorm gain/bias are folded host-side: g into W1 rows / watt_ti rows /
    wfp rows (trunk carries t + ffn/g, i.e. the pre-gain stream); the constant
    c = b_ln + b2 folds into downstream attention/fp biases.  On-chip LN apply
    is only (r - mean)*rstd.
  - Attention + fused projection stay bf16 (their fp8 noise would land
    directly on the output; they are only ~1/3 of the MACs).
  - Data-parallel over batch: 16384 rows -> 8 cores x 2048.
  - Software pipeline with 1-strip skew; LN broadcast+apply for strip s-1 is
    emitted mid-attention of strip s so the PE never waits on the LN row-stat
    chain (which would also re-throttle the PE clock via HAM).
"""

import numpy as np
import ml_dtypes

import concourse.bass as bass
from concourse import bacc, tile, mybir
from concourse.bass_utils import run_bass_kernel_spmd

BF16 = ml_dtypes.bfloat16
F8 = ml_dtypes.float8_e4m3
DT_BF = mybir.dt.bfloat16
DT_F8 = mybir.dt.float8e4
DT_F32 = mybir.dt.float32
AF = mybir.ActivationFunctionType
ALU = mybir.AluOpType
DR = mybir.MatmulPerfMode.DoubleRow

B_FULL, E, H = 16384, 768, 8
F = 4 * E  # 3072
N_CORES = 8
BS = B_FULL // N_CORES  # 2048
EPS = 1e-5
P = 128
KE = E // P  # 6
KF = F // P  # 24
SW = 256.0   # fp8 weight scale
SX = 16.0    # fp8 activation scale


def build(bs=BS, strip=512, sub=512, use_gelu=True, num_devices=N_CORES,
          sim_safe=False):
    """Emit the per-core Bass program (SPMD: same program on every core)."""
    nstrip = bs // strip
    nsub = strip // sub
    assert nstrip * strip == bs and nsub * sub == strip

    nc = bacc.Bacc(
        "TRN2", target_bir_lowering=False, debug=False, num_devices=num_devices
    )

    # ---- DRAM I/O ----
    d_img = nc.dram_tensor("imageT", [E, bs], DT_BF, kind="ExternalInput")
    d_txt = nc.dram_tensor("textT", [E, bs], DT_BF, kind="ExternalInput")
    d_watt = {
        "it": nc.dram_tensor("watt_it", [E, E], DT_BF, kind="ExternalInput"),
        "ti": nc.dram_tensor("watt_ti", [E, E], DT_BF, kind="ExternalInput"),
    }
    d_wfp = nc.dram_tensor("wfp", [2 * E, E], DT_BF, kind="ExternalInput")
    d_w1 = {
        p: nc.dram_tensor(f"w1_{p}", [E, F], DT_F8, kind="ExternalInput")
        for p in ("fi", "ft")
    }
    d_w2 = {
        p: nc.dram_tensor(f"w2_{p}", [F, E], DT_F8, kind="ExternalInput")
        for p in ("fi", "ft")
    }
    bias_specs = {
        "batt_it": KE, "b1_fi": KF,
        "batt_ti": KE, "b1_ft": KF,
        "bfp": KE, "g_fp": KE, "b_fp_ln": KE,
    }
    d_bias = {
        n: nc.dram_tensor(n, [P, k], DT_F32, kind="ExternalInput")
        for n, k in bias_specs.items()
    }
    d_out = nc.dram_tensor("outT", [E, bs], DT_F32, kind="ExternalOutput")

    def dview(d):  # [E|2E, bs] dram -> [p, kt, n] view
        return d.ap().rearrange("(kt p) n -> p kt n", p=P)

    with tile.TileContext(nc) as tc:
        from contextlib import ExitStack

        with ExitStack() as ctx:
            const = ctx.enter_context(tc.tile_pool(name="const", bufs=1))
            pin = ctx.enter_context(tc.tile_pool(name="pin", bufs=2))
            pwork = ctx.enter_context(tc.tile_pool(name="pwork", bufs=2))
            ph = ctx.enter_context(tc.tile_pool(name="ph", bufs=1))
            prow = ctx.enter_context(tc.tile_pool(name="prow", bufs=1))
            pst = ctx.enter_context(tc.tile_pool(name="pst", bufs=3))
            pps = ctx.enter_context(
                tc.tile_pool(name="pps", bufs=2, space=bass.MemorySpace.PSUM)
            )
            pdram = ctx.enter_context(
                tc.tile_pool(name="pdram", bufs=1, space=bass.MemorySpace.DRAM)
            )

            # ---- constants needed for SP1 start (small, DMA'd first) ----
            ones_sb = const.tile([P, P], DT_BF)
            nc.vector.memset(ones_sb[:], 1.0)
            eps_sb = const.tile([1, 1], DT_F32)
            nc.vector.memset(eps_sb[:], EPS)
            watt_sb = {
                pfx: const.tile(
                    [P, KE, E], DT_BF, tag=f"watt_{pfx}", name=f"watt_{pfx}"
                )
                for pfx in ("it", "ti")
            }
            for k in range(KE):
                nc.sync.dma_start(
                    watt_sb["it"][:, k, :], dview(d_watt["it"])[:, k, :]
                )
            bias_sb = {}
            for n, k in bias_specs.items():
                t = const.tile([P, k], DT_F32, tag=f"bias_{n}")
                nc.sync.dma_start(t[:], d_bias[n].ap())
                bias_sb[n] = t
            wfp_sb = const.tile([P, 2 * KE, E], DT_BF)
            # (watt_ti / wfp DMAs are emitted at SP2/SP3 start, see below)

            # ---- internal DRAM trunk: per-strip tiles for fine-grained deps --
            d_img2 = [
                pdram.tile([P, KE, strip], DT_BF, tag=f"img2_{s}", name=f"img2_{s}")
                for s in range(nstrip)
            ]
            d_txt2 = [
                pdram.tile([P, KE, strip], DT_BF, tag=f"txt2_{s}", name=f"txt2_{s}")
                for s in range(nstrip)
            ]

            # ---------- helpers ----------
            def load_strip_ext(dsrc, sl, tag):
                t = pin.tile([P, KE, strip], DT_BF, tag=tag, name=f"in_{tag}")
                nc.sync.dma_start(t[:], dsrc[:, :, sl])
                return t

            def load_strip_trunk(dtile, tag):
                t = pin.tile([P, KE, strip], DT_BF, tag=tag, name=f"in_{tag}")
                nc.sync.dma_start(t[:], dtile[:])
                return t

            def dense_att(rhs_t, resid_t, w_sb, b_sb, mid_hook=None):
                """r[m] = (x @ Wc.T).T[m] + b[m] + resid[m]  (bf16 out)."""
                r = pwork.tile([P, KE, strip], DT_BF, tag="r1", name="r1")
                for m in range(KE):
                    ps = pps.tile([P, strip], DT_F32, tag="mm", bufs=4, name="ps")
                    for k in range(KE):
                        nc.tensor.matmul(
                            ps[:],
                            w_sb[:, k, m * P:(m + 1) * P],
                            rhs_t[:, k, :],
                            start=(k == 0),
                            stop=(k == KE - 1),
                        )
                    nc.vector.scalar_tensor_tensor(
                        r[:, m, :], ps[:], b_sb[:, m:m + 1], resid_t[:, m, :],
                        ALU.add, ALU.add,
                    )
                    if m == 3 and mid_hook is not None:
                        mid_hook()
                return r

            def ln_presum(r):
                """DVE feature pre-sums of r and r^2 -> [P,strip] bf16 pair."""
                s = pwork.tile([P, strip], DT_BF, tag="s", name="s")
                nc.vector.tensor_tensor(s[:], r[:, 0, :], r[:, 1, :], ALU.add)
                for k in range(2, KE):
                    nc.vector.tensor_tensor(s[:], s[:], r[:, k, :], ALU.add)
                sq = pwork.tile([P, strip], DT_BF, tag="sq", name="sq")
                tmp = pwork.tile([P, strip], DT_BF, tag="sqtmp", name="sqtmp")
                nc.vector.tensor_tensor(sq[:], r[:, 0, :], r[:, 0, :], ALU.mult)
                for k in range(1, KE):
                    nc.vector.tensor_tensor(tmp[:], r[:, k, :], r[:, k, :], ALU.mult)
                    nc.vector.tensor_tensor(sq[:], sq[:], tmp[:], ALU.add)
                return s, sq

            def ln_redrows(ssq):
                """PE partition-reduce + row-stat chain -> (mean, rstd) rows."""
                s, sq = ssq
                red0 = pps.tile([1, strip], DT_F32, tag="hps", bufs=2, name="red0")
                red1 = pps.tile([1, strip], DT_F32, tag="ops", bufs=2, name="red1")
                nc.tensor.matmul(red0[:], ones_sb[:, 0:1], s[:], start=True, stop=True)
                nc.tensor.matmul(red1[:], ones_sb[:, 0:1], sq[:], start=True, stop=True)
                mean_bf = prow.tile([1, strip], DT_BF, tag="mean", name="mean")
                nc.scalar.activation(mean_bf[:], red0[:], AF.Copy, scale=1.0 / E)
                msq = prow.tile([1, strip], DT_F32, tag="msq", name="msq")
                nc.vector.tensor_tensor(msq[:], mean_bf[:], mean_bf[:], ALU.mult)
                var = prow.tile([1, strip], DT_F32, tag="var", name="var")
                nc.vector.scalar_tensor_tensor(
                    var[:], red1[:], 1.0 / E, msq[:], ALU.mult, ALU.subtract
                )
                rstd_bf = prow.tile([1, strip], DT_BF, tag="rstdbf", name="rstdbf")
                if sim_safe:
                    std = prow.tile([1, strip], DT_F32, tag="std", name="std")
                    nc.scalar.activation(std[:], var[:], AF.Sqrt, bias=eps_sb[0:1, 0:1])
                    rstd = prow.tile([1, strip], DT_F32, tag="rstd", name="rstd")
                    nc.vector.reciprocal(rstd[:], std[:])
                    nc.vector.tensor_copy(rstd_bf[:], rstd[:])
                else:
                    nc.scalar.activation(
                        rstd_bf[:], var[:], AF.Abs_reciprocal_sqrt,
                        bias=eps_sb[0:1, 0:1],
                    )
                return mean_bf, rstd_bf

            def ln_bcast(rows):
                """PE K=1 broadcast of stats + ACT copy to SBUF bf16 pair."""
                mean_bf, rstd_bf = rows
                mb = pps.tile([P, strip], DT_F32, tag="hps", bufs=2, name="mb")
                nc.tensor.matmul(mb[:], ones_sb[0:1, :], mean_bf[:], start=True, stop=True)
                rb = pps.tile([P, strip], DT_F32, tag="ops", bufs=2, name="rb")
                nc.tensor.matmul(rb[:], ones_sb[0:1, :], rstd_bf[:], start=True, stop=True)
                mbS = pwork.tile([P, strip], DT_BF, tag="mbS", name="mbS")
                nc.scalar.activation(mbS[:], mb[:], AF.Copy)
                rbS = pwork.tile([P, strip], DT_BF, tag="rbS", name="rbS")
                nc.scalar.activation(rbS[:], rb[:], AF.Copy)
                return mbS, rbS

            def ln_apply(r, rows, want_fp8):
                """t = (r - mean)*rstd (bf16 [P,KE,strip]); t8 = fp8(SX*t)."""
                mbS, rbS = ln_bcast(rows)
                t = pwork.tile([P, KE, strip], DT_BF, tag="t", name="t")
                t8 = (
                    pwork.tile([P, KE, strip], DT_F8, tag="t8", name="t8")
                    if want_fp8 else None
                )
                tb = pwork.tile([P, strip], DT_BF, tag="tb", name="tb")
                for k in range(KE):
                    nc.vector.tensor_tensor(tb[:], r[:, k, :], mbS[:], ALU.subtract)
                    nc.vector.tensor_tensor(t[:, k, :], tb[:], rbS[:], ALU.mult)
                    if want_fp8:
                        nc.vector.tensor_scalar_mul(t8[:, k, :], t[:, k, :], SX)
                return t, t8

            def ffn(t, t8, w1, w2, b1_sb, dtile):
                """dtile[:, m, :] = t + (gelu-ffn in fp8 DoubleRow)/SW."""
                for si in range(nsub):
                    ssl = slice(si * sub, (si + 1) * sub)
                    h8 = ph.tile([P, KF, sub], DT_F8, tag="h", name="h")
                    for m in range(KF):
                        hps = pps.tile([P, sub], DT_F32, tag="hps", bufs=2, name="hps")
                        for j in range(KE // 2):
                            nc.tensor.matmul(
                                hps[:],
                                w1[:, 2 * j:2 * j + 2, m * P:(m + 1) * P],
                                t8[:, 2 * j:2 * j + 2, ssl],
                                start=(j == 0), stop=(j == KE // 2 - 1),
                                perf_mode=DR,
                            )
                        nc.scalar.activation(
                            h8[:, m, :], hps[:],
                            AF.Gelu if use_gelu else AF.Identity,
                            bias=b1_sb[:, m:m + 1], scale=1.0 / (SX * SW),
                        )
                    for m in range(KE):
                        ops = pps.tile([P, sub], DT_F32, tag="ops", bufs=2, name="ops")
                        for j in range(KF // 2):
                            nc.tensor.matmul(
                                ops[:],
                                w2[:, 2 * j:2 * j + 2, m * P:(m + 1) * P],
                                h8[:, 2 * j:2 * j + 2, :],
                                start=(j == 0), stop=(j == KF // 2 - 1),
                                perf_mode=DR,
                            )
                        st = pst.tile([P, sub], DT_BF, tag="stg", name="stg")
                        nc.vector.scalar_tensor_tensor(
                            st[:], ops[:], 1.0 / SW, t[:, m, ssl],
                            ALU.mult, ALU.add,
                        )
                        nc.sync.dma_start(dtile[:, m, ssl], st[:])

            def superphase(pfx, rhs_src, res_view, watt, b_att, w1d, w2d, b1,
                           dout):
                """rhs_src: callable s -> rhs strip tile; res_view: dram view."""
                with tc.tile_pool(name=f"wffn_{pfx}", bufs=1) as wp:
                    w1 = wp.tile([P, KE, F], DT_F8, tag="w1", name="w1")
                    w2 = wp.tile([P, KF, E], DT_F8, tag="w2", name="w2")

                    def load_w():
                        for k in range(KE):
                            nc.sync.dma_start(
                                w1[:, k, :], w1d.ap()[k * P:(k + 1) * P, :]
                            )
                        for k in range(KF):
                            nc.sync.dma_start(
                                w2[:, k, :], w2d.ap()[k * P:(k + 1) * P, :]
                            )

                    pend = None  # (r, rows, strip idx) awaiting bcast/apply+ffn
                    xcur = [None]
                    for s in range(nstrip):
                        sl = slice(s * strip, (s + 1) * strip)
                        rhs_t = rhs_src(s)
                        res_t = load_strip_ext(res_view, sl, "res_in")

                        hook = None
                        if pend is not None:
                            rp, rowsp, _ = pend

                            def hook(rp=rp, rowsp=rowsp):
                                xcur[0] = ln_apply(rp, rowsp, want_fp8=True)

                        r = dense_att(rhs_t, res_t, watt, b_att, mid_hook=hook)
                        ssq = ln_presum(r)
                        if s == 0 and pfx == "fi":
                            load_w()  # after the first strip's work is queued
                            for k in range(KE):
                                nc.sync.dma_start(
                                    watt_sb["ti"][:, k, :],
                                    dview(d_watt["ti"])[:, k, :],
                                )
                        if pend is not None:
                            _, _, sp = pend
                            ffn(xcur[0][0], xcur[0][1], w1, w2, b1, dout[sp])
                        if s == 0 and pfx != "fi":
                            load_w()
                            for k in range(2 * KE):
                                nc.sync.dma_start(
                                    wfp_sb[:, k, :], dview(d_wfp)[:, k, :]
                                )
                        rows = ln_redrows(ssq)
                        pend = (r, rows, s)
                    rp, rowsp, sp = pend
                    t, t8 = ln_apply(rp, rowsp, want_fp8=True)
                    ffn(t, t8, w1, w2, b1, dout[sp])

            # ---- SP1: image branch (kv = text, residual = image) ----
            superphase(
                "fi",
                lambda s: load_strip_ext(
                    dview(d_txt), slice(s * strip, (s + 1) * strip), "rhs_in"
                ),
                dview(d_img), watt_sb["it"], bias_sb["batt_it"],
                d_w1["fi"], d_w2["fi"], bias_sb["b1_fi"],
                d_img2,
            )
            # ---- SP2: text branch (kv = img2, residual = text) ----
            superphase(
                "ft",
                lambda s: load_strip_trunk(d_img2[s], "rhs_in"),
                dview(d_txt), watt_sb["ti"], bias_sb["batt_ti"],
                d_w1["ft"], d_w2["ft"], bias_sb["b1_ft"],
                d_txt2,
            )

            # ---- SP3: fused projection + LN + gelu ----
            with tc.tile_pool(name="sp3", bufs=3) as p3:
                outv = dview(d_out)

                def fp_finish(rp3, slp, rowsp):
                    mbS, rbS = ln_bcast(rowsp)
                    tb3 = p3.tile([P, strip], DT_BF, tag="tb3", name="tb3")
                    t3 = p3.tile([P, strip], DT_BF, tag="t3", name="t3")
                    for k in range(KE):
                        nc.vector.tensor_tensor(
                            tb3[:], rp3[:, k, :], mbS[:], ALU.subtract
                        )
                        nc.vector.tensor_tensor(t3[:], tb3[:], rbS[:], ALU.mult)
                        o = p3.tile([P, strip], DT_F32, tag="of32", name="of32")
                        nc.scalar.activation(
                            o[:], t3[:],
                            AF.Gelu if use_gelu else AF.Identity,
                            bias=bias_sb["b_fp_ln"][:, k:k + 1],
                            scale=bias_sb["g_fp"][:, k:k + 1],
                        )
                        nc.sync.dma_start(outv[:, k, slp], o[:])

                # 1-strip skew with in-loop hooks: red(s-1) after m1,
                # finish(s-1) after m5 -- PE never waits on the LN chain.
                stages = []  # per strip dict: r3, sl, ssq, rows
                for s in range(nstrip):
                    sl = slice(s * strip, (s + 1) * strip)
                    a_in = load_strip_trunk(d_img2[s], "rhs_in")
                    b_in = load_strip_trunk(d_txt2[s], "res_in")
                    r3 = pwork.tile([P, KE, strip], DT_BF, tag="r1", name="r3")
                    for m in range(KE):
                        zps = pps.tile([P, strip], DT_F32, tag="mm", bufs=4, name="zps")
                        for k in range(2 * KE):
                            src = a_in if k < KE else b_in
                            nc.tensor.matmul(
                                zps[:], wfp_sb[:, k, m * P:(m + 1) * P],
                                src[:, k % KE, :],
                                start=(k == 0), stop=(k == 2 * KE - 1),
                            )
                        nc.scalar.activation(
                            r3[:, m, :], zps[:], AF.Identity,
                            bias=bias_sb["bfp"][:, m:m + 1],
                        )
                        if m == 1 and stages and "rows" not in stages[-1]:
                            stages[-1]["rows"] = ln_redrows(stages[-1]["ssq"])
                        if m == 5 and stages and not stages[-1].get("done"):
                            st1 = stages[-1]
                            fp_finish(st1["r3"], st1["sl"], st1["rows"])
                            st1["done"] = True
                    stages.append({"r3": r3, "sl": sl, "ssq": ln_presum(r3)})
                last = stages[-1]
                last["rows"] = ln_redrows(last["ssq"])
                fp_finish(last["r3"], last["sl"], last["rows"])

    nc.compile()
    return nc


# ---------------- host side ----------------

_BUILT = {}


def _get_nc(key):
    if key not in _BUILT:
        _BUILT[key] = build(*key)
    return _BUILT[key]


def _packv(v, ktiles):
    return np.ascontiguousarray(np.asarray(v, np.float32).reshape(ktiles, P).T)


def prep_inputs(inputs, bs=BS, n_cores=N_CORES):
    f32 = np.float32
    g = lambda n: np.asarray(inputs[n], f32)
    g_img, b_img = g("ln_img_g"), g("ln_img_b")
    g_txt, b_txt = g("ln_text_g"), g("ln_text_b")
    c_img = b_img + g("fi_b2")  # constant the img trunk omits
    c_txt = b_txt + g("ft_b2")
    common = {}
    # --- attention (bf16): Wc = Wo@Wv; ti's rows absorb g_img, bias absorbs
    # the img trunk's missing constant c_img.
    wc_it = g("it_Wo") @ g("it_Wv")
    bc_it = g("it_Wo") @ g("it_bv") + g("it_bo")
    common["watt_it"] = np.ascontiguousarray(wc_it.T).astype(BF16)
    common["batt_it"] = _packv(bc_it, KE)
    wc_ti = g("ti_Wo") @ g("ti_Wv")
    bc_ti = g("ti_Wo") @ g("ti_bv") + g("ti_bo") + wc_ti @ c_img
    common["watt_ti"] = np.ascontiguousarray(wc_ti.T * g_img[:, None]).astype(BF16)
    common["batt_ti"] = _packv(bc_ti, KE)
    # --- FFN (fp8): W1 rows absorb g_ln (input is t, not x); W2 columns are
    # divided by g_ln (trunk carries t + ffn/g); biases b1 absorb b_ln@W1.T.
    for p, gl, bl in (("fi", g_img, b_img), ("ft", g_txt, b_txt)):
        w1 = g(f"{p}_W1")  # [F, E]
        w2 = g(f"{p}_W2")  # [E, F]
        common[f"w1_{p}"] = np.ascontiguousarray(
            w1.T * (gl[:, None] * SW)).astype(F8)
        common[f"w2_{p}"] = np.ascontiguousarray(
            w2.T * (SW / gl[None, :])).astype(F8)
        common[f"b1_{p}"] = _packv(g(f"{p}_b1") + w1 @ bl, KF)
    # --- fused projection (bf16): rows absorb [g_img; g_txt]; bias absorbs
    # the trunks' missing constants.
    fpw = g("fp_W")  # [E, 2E]
    g_cat = np.concatenate([g_img, g_txt])
    c_cat = np.concatenate([c_img, c_txt])
    common["wfp"] = np.ascontiguousarray(fpw.T * g_cat[:, None]).astype(BF16)
    common["bfp"] = _packv(g("fp_b") + fpw @ c_cat, KE)
    common["g_fp"] = _packv(g("fp_ln_g"), KE)
    common["b_fp_ln"] = _packv(g("fp_ln_b"), KE)

    imgT = g("image_embed").T.astype(BF16)
    txtT = g("text_embed").T.astype(BF16)
    in_maps = []
    for c in range(n_cores):
        sl = slice(c * bs, (c + 1) * bs)
        m = dict(common)
        m["imageT"] = np.ascontiguousarray(imgT[:, sl])
        m["textT"] = np.ascontiguousarray(txtT[:, sl])
        in_maps.append(m)
    return in_maps


CFG = (BS, 512, 512, True, N_CORES)


def kernel(**inputs):
    nc = _get_nc(CFG)
    in_maps = prep_inputs(inputs)
    res = run_bass_kernel_spmd(nc, in_maps, core_ids=list(range(N_CORES)))
    out = np.concatenate(
        [res.results[c]["outT"] for c in range(N_CORES)], axis=1
    )  # [E, B]
    return np.ascontiguousarray(out.T).astype(np.float32)


# revision 5
# speedup vs baseline: 1.4034x; 1.0161x over previous
"""Trainium2 Bass kernel: CrossAttentionFusion (dense transformer block pair).

Math notes (vs the reference):
  - seq_len-1 cross attention: softmax over a single key == 1, so
    mha1(q_in, kv_in) == kv_in @ (Wo@Wv).T + (Wo@bv + bo).  q/k projections are
    dead code; the two projections fuse into ONE 768x768 matmul (host-fused).
  - Transposed layout: activations live as [feature, batch]; matmuls are
    lhsT(=W.T, stationary) x rhs(=x.T, moving) -> out = (x@W.T).T.
    LayerNorm reduces over features (= partitions) with a ones-vector matmul on
    the PE; per-sample stats are broadcast back over partitions with K=1 ones
    matmuls.
  - FFN matmuls run in fp8e4 (e4m3) with MatmulPerfMode.DoubleRow: one PE
    instruction contracts TWO 128-row k-subtiles (2x bf16 FLOP rate).  Weights
    are host-quantized with a 256x scale (so sigma~0.02 values sit in e4m3's
    normal range); activations are quantized on the DVE (t8 = 16*t) and by the
    gelu ACT op writing fp8 directly (h8).  Scales unwind in the ACT/DVE
    epilogues (gelu scale = 1/(16*256); FFN2 out scale = 1/256).
  - LayerNorm gain/bias are folded host-side: g into W1 rows / watt_ti rows /
    wfp rows (trunk carries t + ffn/g, i.e. the pre-gain stream); the constant
    c = b_ln + b2 folds into downstream attention/fp biases.  On-chip LN apply
    is only (r - mean)*rstd.
  - Attention + fused projection stay bf16 (their fp8 noise would land
    directly on the output; they are only ~1/3 of the MACs).
  - Data-parallel over batch: 16384 rows -> 8 cores x 2048.
  - Software pipeline with 1-strip skew; LN broadcast+apply for strip s-1 is
    emitted mid-attention of strip s so the PE never waits on the LN row-stat
    chain (which would also re-throttle the PE clock via HAM).
"""

import numpy as np
import ml_dtypes

import concourse.bass as bass
from concourse import bacc, tile, mybir
from concourse.bass_utils import run_bass_kernel_spmd

BF16 = ml_dtypes.bfloat16
F8 = ml_dtypes.float8_e4m3
DT_BF = mybir.dt.bfloat16
DT_F8 = mybir.dt.float8e4
DT_F32 = mybir.dt.float32
AF = mybir.ActivationFunctionType
ALU = mybir.AluOpType
DR = mybir.MatmulPerfMode.DoubleRow

B_FULL, E, H = 16384, 768, 8
F = 4 * E  # 3072
N_CORES = 8
BS = B_FULL // N_CORES  # 2048
EPS = 1e-5
P = 128
KE = E // P  # 6
KF = F // P  # 24
SW = 256.0   # fp8 weight scale
SX = 16.0    # fp8 activation scale


def build(bs=BS, strip=512, sub=512, use_gelu=True, num_devices=N_CORES,
          sim_safe=False):
    """Emit the per-core Bass program (SPMD: same program on every core)."""
    nstrip = bs // strip
    nsub = strip // sub
    assert nstrip * strip == bs and nsub * sub == strip

    nc = bacc.Bacc(
        "TRN2", target_bir_lowering=False, debug=False, num_devices=num_devices
    )

    # ---- DRAM I/O ----
    d_img = nc.dram_tensor("imageT", [E, bs], DT_BF, kind="ExternalInput")
    d_txt = nc.dram_tensor("textT", [E, bs], DT_BF, kind="ExternalInput")
    d_watt = {
        "it": nc.dram_tensor("watt_it", [E, E], DT_BF, kind="ExternalInput"),
        "ti": nc.dram_tensor("watt_ti", [E, E], DT_BF, kind="ExternalInput"),
    }
    d_wfp = nc.dram_tensor("wfp", [2 * E, E], DT_BF, kind="ExternalInput")
    d_w1 = {
        p: nc.dram_tensor(f"w1_{p}", [E, F], DT_F8, kind="ExternalInput")
        for p in ("fi", "ft")
    }
    d_w2 = {
        p: nc.dram_tensor(f"w2_{p}", [F, E], DT_F8, kind="ExternalInput")
        for p in ("fi", "ft")
    }
    bias_specs = {
        "batt_it": KE, "b1_fi": KF,
        "batt_ti": KE, "b1_ft": KF,
        "bfp": KE, "g_fp": KE, "b_fp_ln": KE,
    }
    d_bias = {
        n: nc.dram_tensor(n, [P, k], DT_F32, kind="ExternalInput")
        for n, k in bias_specs.items()
    }
    d_out = nc.dram_tensor("outT", [E, bs], DT_F32, kind="ExternalOutput")

    def dview(d):  # [E|2E, bs] dram -> [p, kt, n] view
        return d.ap().rearrange("(kt p) n -> p kt n", p=P)

    with tile.TileContext(nc) as tc:
        from contextlib import ExitStack

        with ExitStack() as ctx:
            const = ctx.enter_context(tc.tile_pool(name="const", bufs=1))
            pin = ctx.enter_context(tc.tile_pool(name="pin", bufs=2))
            pwork = ctx.enter_context(tc.tile_pool(name="pwork", bufs=2))
            ph = ctx.enter_context(tc.tile_pool(name="ph", bufs=1))
            prow = ctx.enter_context(tc.tile_pool(name="prow", bufs=1))
            pst = ctx.enter_context(tc.tile_pool(name="pst", bufs=3))
            pps = ctx.enter_context(
                tc.tile_pool(name="pps", bufs=2, space=bass.MemorySpace.PSUM)
            )
            pdram = ctx.enter_context(
                tc.tile_pool(name="pdram", bufs=1, space=bass.MemorySpace.DRAM)
            )

            # ---- constants needed for SP1 start (small, DMA'd first) ----
            ones_sb = const.tile([P, P], DT_BF)
            nc.vector.memset(ones_sb[:], 1.0)
            eps_sb = const.tile([1, 1], DT_F32)
            nc.vector.memset(eps_sb[:], EPS)
            watt_sb = {
                pfx: const.tile(
                    [P, KE, E], DT_BF, tag=f"watt_{pfx}", name=f"watt_{pfx}"
                )
                for pfx in ("it", "ti")
            }
            for k in range(KE):
                nc.sync.dma_start(
                    watt_sb["it"][:, k, :], dview(d_watt["it"])[:, k, :]
                )
            bias_sb = {}
            for n, k in bias_specs.items():
                t = const.tile([P, k], DT_F32, tag=f"bias_{n}")
                nc.sync.dma_start(t[:], d_bias[n].ap())
                bias_sb[n] = t
            wfp_sb = const.tile([P, 2 * KE, E], DT_BF)
            # (watt_ti / wfp DMAs are emitted at SP2/SP3 start, see below)

            # ---- internal DRAM trunk: per-strip tiles for fine-grained deps --
            d_img2 = [
                pdram.tile([P, KE, strip], DT_BF, tag=f"img2_{s}", name=f"img2_{s}")
                for s in range(nstrip)
            ]
            d_txt2 = [
                pdram.tile([P, KE, strip], DT_BF, tag=f"txt2_{s}", name=f"txt2_{s}")
                for s in range(nstrip)
            ]

            # ---------- helpers ----------
            def load_strip_ext(dsrc, sl, tag):
                t = pin.tile([P, KE, strip], DT_BF, tag=tag, name=f"in_{tag}")
                nc.sync.dma_start(t[:], dsrc[:, :, sl])
                return t

            def load_strip_trunk(dtile, tag):
                t = pin.tile([P, KE, strip], DT_BF, tag=tag, name=f"in_{tag}")
                nc.sync.dma_start(t[:], dtile[:])
                return t

            def dense_att(rhs_t, resid_t, w_sb, b_sb):
                """r[m] = (x @ Wc.T).T[m] + b[m] + resid[m]  (bf16 out)."""
                r = pwork.tile([P, KE, strip], DT_BF, tag="r1", name="r1")
                for m in range(KE):
                    ps = pps.tile([P, strip], DT_F32, tag="mm", bufs=4, name="ps")
                    for k in range(KE):
                        nc.tensor.matmul(
                            ps[:],
                            w_sb[:, k, m * P:(m + 1) * P],
                            rhs_t[:, k, :],
                            start=(k == 0),
                            stop=(k == KE - 1),
                        )
                    nc.vector.scalar_tensor_tensor(
                        r[:, m, :], ps[:], b_sb[:, m:m + 1], resid_t[:, m, :],
                        ALU.add, ALU.add,
                    )
                return r

            def ln_presum(r):
                """DVE feature pre-sums of r and r^2 -> [P,strip] bf16 pair."""
                s = pwork.tile([P, strip], DT_BF, tag="s", name="s")
                nc.vector.tensor_tensor(s[:], r[:, 0, :], r[:, 1, :], ALU.add)
                for k in range(2, KE):
                    nc.vector.tensor_tensor(s[:], s[:], r[:, k, :], ALU.add)
                sq = pwork.tile([P, strip], DT_BF, tag="sq", name="sq")
                tmp = pwork.tile([P, strip], DT_BF, tag="sqtmp", name="sqtmp")
                nc.vector.tensor_tensor(sq[:], r[:, 0, :], r[:, 0, :], ALU.mult)
                for k in range(1, KE):
                    nc.vector.tensor_tensor(tmp[:], r[:, k, :], r[:, k, :], ALU.mult)
                    nc.vector.tensor_tensor(sq[:], sq[:], tmp[:], ALU.add)
                return s, sq

            def ln_redrows(ssq):
                """PE partition-reduce + row-stat chain -> (mean, rstd) rows."""
                s, sq = ssq
                red0 = pps.tile([1, strip], DT_F32, tag="hps", bufs=2, name="red0")
                red1 = pps.tile([1, strip], DT_F32, tag="ops", bufs=2, name="red1")
                nc.tensor.matmul(red0[:], ones_sb[:, 0:1], s[:], start=True, stop=True)
                nc.tensor.matmul(red1[:], ones_sb[:, 0:1], sq[:], start=True, stop=True)
                mean_bf = prow.tile([1, strip], DT_BF, tag="mean", name="mean")
                nc.scalar.activation(mean_bf[:], red0[:], AF.Copy, scale=1.0 / E)
                msq = prow.tile([1, strip], DT_F32, tag="msq", name="msq")
                nc.vector.tensor_tensor(msq[:], mean_bf[:], mean_bf[:], ALU.mult)
                var = prow.tile([1, strip], DT_F32, tag="var", name="var")
                nc.vector.scalar_tensor_tensor(
                    var[:], red1[:], 1.0 / E, msq[:], ALU.mult, ALU.subtract
                )
                rstd_bf = prow.tile([1, strip], DT_BF, tag="rstdbf", name="rstdbf")
                if sim_safe:
                    std = prow.tile([1, strip], DT_F32, tag="std", name="std")
                    nc.scalar.activation(std[:], var[:], AF.Sqrt, bias=eps_sb[0:1, 0:1])
                    rstd = prow.tile([1, strip], DT_F32, tag="rstd", name="rstd")
                    nc.vector.reciprocal(rstd[:], std[:])
                    nc.vector.tensor_copy(rstd_bf[:], rstd[:])
                else:
                    nc.scalar.activation(
                        rstd_bf[:], var[:], AF.Abs_reciprocal_sqrt,
                        bias=eps_sb[0:1, 0:1],
                    )
                return mean_bf, rstd_bf

            def ln_bcast(rows):
                """PE K=1 broadcast of stats + ACT copy to SBUF bf16 pair."""
                mean_bf, rstd_bf = rows
                mb = pps.tile([P, strip], DT_F32, tag="hps", bufs=2, name="mb")
                nc.tensor.matmul(mb[:], ones_sb[0:1, :], mean_bf[:], start=True, stop=True)
                rb = pps.tile([P, strip], DT_F32, tag="ops", bufs=2, name="rb")
                nc.tensor.matmul(rb[:], ones_sb[0:1, :], rstd_bf[:], start=True, stop=True)
                mbS = pwork.tile([P, strip], DT_BF, tag="mbS", name="mbS")
                nc.scalar.activation(mbS[:], mb[:], AF.Copy)
                rbS = pwork.tile([P, strip], DT_BF, tag="rbS", name="rbS")
                nc.scalar.activation(rbS[:], rb[:], AF.Copy)
                return mbS, rbS

            def ln_apply(r, rows, want_fp8):
                """t = (r - mean)*rstd (bf16 [P,KE,strip]); t8 = fp8(SX*t)."""
                mbS, rbS = ln_bcast(rows)
                t = pwork.tile([P, KE, strip], DT_BF, tag="t", name="t")
                t8 = (
                    pwork.tile([P, KE, strip], DT_F8, tag="t8", name="t8")
                    if want_fp8 else None
                )
                tb = pwork.tile([P, strip], DT_BF, tag="tb", name="tb")
                for k in range(KE):
                    nc.vector.tensor_tensor(tb[:], r[:, k, :], mbS[:], ALU.subtract)
                    nc.vector.tensor_tensor(t[:, k, :], tb[:], rbS[:], ALU.mult)
                    if want_fp8:
                        nc.vector.tensor_scalar_mul(t8[:, k, :], t[:, k, :], SX)
                return t, t8

            def ffn(t, t8, w1, w2, b1_sb, dtile, mid_hook=None):
                """dtile[:, m, :] = t + (gelu-ffn in fp8 DoubleRow)/SW.

                mid_hook (if given) fires after FFN2 m==1: the ACT engine is
                idle during FFN2 (st' epilogues are DVE), so the LN row-stat
                chain + its ACT table switches land off the gelu stream."""
                for si in range(nsub):
                    ssl = slice(si * sub, (si + 1) * sub)
                    h8 = ph.tile([P, KF, sub], DT_F8, tag="h", name="h")
                    for m in range(KF):
                        hps = pps.tile([P, sub], DT_F32, tag="hps", bufs=2, name="hps")
                        for j in range(KE // 2):
                            nc.tensor.matmul(
                                hps[:],
                                w1[:, 2 * j:2 * j + 2, m * P:(m + 1) * P],
                                t8[:, 2 * j:2 * j + 2, ssl],
                                start=(j == 0), stop=(j == KE // 2 - 1),
                                perf_mode=DR,
                            )
                        nc.scalar.activation(
                            h8[:, m, :], hps[:],
                            AF.Gelu if use_gelu else AF.Identity,
                            bias=b1_sb[:, m:m + 1], scale=1.0 / (SX * SW),
                        )
                    for m in range(KE):
                        ops = pps.tile([P, sub], DT_F32, tag="ops", bufs=2, name="ops")
                        for j in range(KF // 2):
                            nc.tensor.matmul(
                                ops[:],
                                w2[:, 2 * j:2 * j + 2, m * P:(m + 1) * P],
                                h8[:, 2 * j:2 * j + 2, :],
                                start=(j == 0), stop=(j == KF // 2 - 1),
                                perf_mode=DR,
                            )
                        st = pst.tile([P, sub], DT_BF, tag="stg", name="stg")
                        nc.vector.scalar_tensor_tensor(
                            st[:], ops[:], 1.0 / SW, t[:, m, ssl],
                            ALU.mult, ALU.add,
                        )
                        nc.sync.dma_start(dtile[:, m, ssl], st[:])
                        if m == 1 and si == nsub - 1 and mid_hook is not None:
                            mid_hook()

            def superphase(pfx, rhs_src, res_view, watt, b_att, w1d, w2d, b1,
                           dout):
                """rhs_src: callable s -> rhs strip tile; res_view: dram view."""
                with tc.tile_pool(name=f"wffn_{pfx}", bufs=1) as wp:
                    w1 = wp.tile([P, KE, F], DT_F8, tag="w1", name="w1")
                    w2 = wp.tile([P, KF, E], DT_F8, tag="w2", name="w2")

                    def load_w():
                        for k in range(KE):
                            nc.sync.dma_start(
                                w1[:, k, :], w1d.ap()[k * P:(k + 1) * P, :]
                            )
                        for k in range(KF):
                            nc.sync.dma_start(
                                w2[:, k, :], w2d.ap()[k * P:(k + 1) * P, :]
                            )

                    pend = None  # (r, rows, strip idx) awaiting apply+ffn
                    for s in range(nstrip):
                        sl = slice(s * strip, (s + 1) * strip)
                        rhs_t = rhs_src(s)
                        res_t = load_strip_ext(res_view, sl, "res_in")

                        # apply(s-1) at iter top: the whole of att(s) covers
                        # its PE-bcast + ACT-copy + DVE chain.
                        cur = None
                        if pend is not None:
                            rp, rowsp, _ = pend
                            cur = ln_apply(rp, rowsp, want_fp8=True)

                        r = dense_att(rhs_t, res_t, watt, b_att)
                        ssq = ln_presum(r)
                        if s == 0 and pfx == "fi":
                            load_w()  # after the first strip's work is queued
                            for k in range(KE):
                                nc.sync.dma_start(
                                    watt_sb["ti"][:, k, :],
                                    dview(d_watt["ti"])[:, k, :],
                                )
                        rbox = [None]
                        if pend is not None:
                            _, _, sp = pend
                            ffn(cur[0], cur[1], w1, w2, b1, dout[sp],
                                mid_hook=lambda: rbox.__setitem__(
                                    0, ln_redrows(ssq)))
                        if s == 0 and pfx != "fi":
                            load_w()
                            for k in range(2 * KE):
                                nc.sync.dma_start(
                                    wfp_sb[:, k, :], dview(d_wfp)[:, k, :]
                                )
                        rows = rbox[0] if rbox[0] is not None else ln_redrows(ssq)
                        pend = (r, rows, s)
                    rp, rowsp, sp = pend
                    t, t8 = ln_apply(rp, rowsp, want_fp8=True)
                    ffn(t, t8, w1, w2, b1, dout[sp])

            # ---- SP1: image branch (kv = text, residual = image) ----
            superphase(
                "fi",
                lambda s: load_strip_ext(
                    dview(d_txt), slice(s * strip, (s + 1) * strip), "rhs_in"
                ),
                dview(d_img), watt_sb["it"], bias_sb["batt_it"],
                d_w1["fi"], d_w2["fi"], bias_sb["b1_fi"],
                d_img2,
            )
            # ---- SP2: text branch (kv = img2, residual = text) ----
            superphase(
                "ft",
                lambda s: load_strip_trunk(d_img2[s], "rhs_in"),
                dview(d_txt), watt_sb["ti"], bias_sb["batt_ti"],
                d_w1["ft"], d_w2["ft"], bias_sb["b1_ft"],
                d_txt2,
            )

            # ---- SP3: fused projection + LN + gelu ----
            with tc.tile_pool(name="sp3", bufs=3) as p3:
                outv = dview(d_out)

                def fp_finish(rp3, slp, rowsp):
                    mbS, rbS = ln_bcast(rowsp)
                    tb3 = p3.tile([P, strip], DT_BF, tag="tb3", name="tb3")
                    t3 = p3.tile([P, strip], DT_BF, tag="t3", name="t3")
                    for k in range(KE):
                        nc.vector.tensor_tensor(
                            tb3[:], rp3[:, k, :], mbS[:], ALU.subtract
                        )
                        nc.vector.tensor_tensor(t3[:], tb3[:], rbS[:], ALU.mult)
                        o = p3.tile([P, strip], DT_F32, tag="of32", name="of32")
                        nc.scalar.activation(
                            o[:], t3[:],
                            AF.Gelu if use_gelu else AF.Identity,
                            bias=bias_sb["b_fp_ln"][:, k:k + 1],
                            scale=bias_sb["g_fp"][:, k:k + 1],
                        )
                        nc.sync.dma_start(outv[:, k, slp], o[:])

                # 1-strip skew with in-loop hooks: red(s-1) after m1,
                # finish(s-1) after m5 -- PE never waits on the LN chain.
                stages = []  # per strip dict: r3, sl, ssq, rows
                for s in range(nstrip):
                    sl = slice(s * strip, (s + 1) * strip)
                    a_in = load_strip_trunk(d_img2[s], "rhs_in")
                    b_in = load_strip_trunk(d_txt2[s], "res_in")
                    r3 = pwork.tile([P, KE, strip], DT_BF, tag="r1", name="r3")
                    for m in range(KE):
                        zps = pps.tile([P, strip], DT_F32, tag="mm", bufs=4, name="zps")
                        for k in range(2 * KE):
                            src = a_in if k < KE else b_in
                            nc.tensor.matmul(
                                zps[:], wfp_sb[:, k, m * P:(m + 1) * P],
                                src[:, k % KE, :],
                                start=(k == 0), stop=(k == 2 * KE - 1),
                            )
                        nc.scalar.activation(
                            r3[:, m, :], zps[:], AF.Identity,
                            bias=bias_sb["bfp"][:, m:m + 1],
                        )
                        if m == 1 and stages and "rows" not in stages[-1]:
                            stages[-1]["rows"] = ln_redrows(stages[-1]["ssq"])
                        if m == 5 and stages and not stages[-1].get("done"):
                            st1 = stages[-1]
                            fp_finish(st1["r3"], st1["sl"], st1["rows"])
                            st1["done"] = True
                    stages.append({"r3": r3, "sl": sl, "ssq": ln_presum(r3)})
                last = stages[-1]
                last["rows"] = ln_redrows(last["ssq"])
                fp_finish(last["r3"], last["sl"], last["rows"])

    nc.compile()
    return nc


# ---------------- host side ----------------

_BUILT = {}


def _get_nc(key):
    if key not in _BUILT:
        _BUILT[key] = build(*key)
    return _BUILT[key]


def _packv(v, ktiles):
    return np.ascontiguousarray(np.asarray(v, np.float32).reshape(ktiles, P).T)


def prep_inputs(inputs, bs=BS, n_cores=N_CORES):
    f32 = np.float32
    g = lambda n: np.asarray(inputs[n], f32)
    g_img, b_img = g("ln_img_g"), g("ln_img_b")
    g_txt, b_txt = g("ln_text_g"), g("ln_text_b")
    c_img = b_img + g("fi_b2")  # constant the img trunk omits
    c_txt = b_txt + g("ft_b2")
    common = {}
    # --- attention (bf16): Wc = Wo@Wv; ti's rows absorb g_img, bias absorbs
    # the img trunk's missing constant c_img.
    wc_it = g("it_Wo") @ g("it_Wv")
    bc_it = g("it_Wo") @ g("it_bv") + g("it_bo")
    common["watt_it"] = np.ascontiguousarray(wc_it.T).astype(BF16)
    common["batt_it"] = _packv(bc_it, KE)
    wc_ti = g("ti_Wo") @ g("ti_Wv")
    bc_ti = g("ti_Wo") @ g("ti_bv") + g("ti_bo") + wc_ti @ c_img
    common["watt_ti"] = np.ascontiguousarray(wc_ti.T * g_img[:, None]).astype(BF16)
    common["batt_ti"] = _packv(bc_ti, KE)
    # --- FFN (fp8): W1 rows absorb g_ln (input is t, not x); W2 columns are
    # divided by g_ln (trunk carries t + ffn/g); biases b1 absorb b_ln@W1.T.
    for p, gl, bl in (("fi", g_img, b_img), ("ft", g_txt, b_txt)):
        w1 = g(f"{p}_W1")  # [F, E]
        w2 = g(f"{p}_W2")  # [E, F]
        common[f"w1_{p}"] = np.ascontiguousarray(
            w1.T * (gl[:, None] * SW)).astype(F8)
        common[f"w2_{p}"] = np.ascontiguousarray(
            w2.T * (SW / gl[None, :])).astype(F8)
        common[f"b1_{p}"] = _packv(g(f"{p}_b1") + w1 @ bl, KF)
    # --- fused projection (bf16): rows absorb [g_img; g_txt]; bias absorbs
    # the trunks' missing constants.
    fpw = g("fp_W")  # [E, 2E]
    g_cat = np.concatenate([g_img, g_txt])
    c_cat = np.concatenate([c_img, c_txt])
    common["wfp"] = np.ascontiguousarray(fpw.T * g_cat[:, None]).astype(BF16)
    common["bfp"] = _packv(g("fp_b") + fpw @ c_cat, KE)
    common["g_fp"] = _packv(g("fp_ln_g"), KE)
    common["b_fp_ln"] = _packv(g("fp_ln_b"), KE)

    imgT = g("image_embed").T.astype(BF16)
    txtT = g("text_embed").T.astype(BF16)
    in_maps = []
    for c in range(n_cores):
        sl = slice(c * bs, (c + 1) * bs)
        m = dict(common)
        m["imageT"] = np.ascontiguousarray(imgT[:, sl])
        m["textT"] = np.ascontiguousarray(txtT[:, sl])
        in_maps.append(m)
    return in_maps


CFG = (BS, 512, 512, True, N_CORES)


def kernel(**inputs):
    nc = _get_nc(CFG)
    in_maps = prep_inputs(inputs)
    res = run_bass_kernel_spmd(nc, in_maps, core_ids=list(range(N_CORES)))
    out = np.concatenate(
        [res.results[c]["outT"] for c in range(N_CORES)], axis=1
    )  # [E, B]
    return np.ascontiguousarray(out.T).astype(np.float32)


# revision 6
# speedup vs baseline: 1.4280x; 1.0175x over previous
"""Trainium2 Bass kernel: CrossAttentionFusion (dense transformer block pair).

Math notes (vs the reference):
  - seq_len-1 cross attention: softmax over a single key is identically 1, so
    mha1(q_in, kv_in) == kv_in @ (Wo@Wv).T + (Wo@bv + bo).  q/k projections are
    dead code; the two projections fuse into ONE 768x768 matmul (host-fused).
  - Transposed layout: activations live as [feature, batch]; matmuls are
    lhsT(=W.T, stationary) x rhs(=x.T, moving) -> out = (x@W.T).T.
    LayerNorm reduces over features (= partitions) with a ones-vector matmul on
    the PE; per-sample stats are broadcast back over partitions with K=1 ones
    matmuls.
  - FFN matmuls run in fp8e4 (e4m3) with MatmulPerfMode.DoubleRow: one PE
    instruction contracts TWO 128-row k-subtiles (2x bf16 FLOP rate).  Weights
    are host-quantized with a 256x scale (so sigma~0.02 values sit in e4m3's
    normal range); activations are quantized by ACT ops writing fp8 directly
    (t8 = Copy(16*t), h8 = Gelu out).  Scales unwind in the epilogues.
  - LayerNorm gain/bias are folded host-side: g into W1 rows / watt_ti rows /
    wfp rows (trunk carries t + ffn/g, i.e. the pre-gain stream); the constant
    c = b_ln + b2 folds into downstream attention/fp biases.  On-chip LN apply
    is only (r - mean)*rstd.
  - Attention + fused projection stay bf16 (their fp8 noise would land
    directly on the output and blow the 2e-2 budget; they are ~1/3 of MACs).
  - Data-parallel over batch: 16384 rows -> 8 cores x 2048.
  - Strips of [256,512,512,512,256]: small edge strips halve the un-hideable
    pipeline fill (first-strip LN chain) and drain (last-strip epilogue).
  - Software pipeline, 1-strip skew: ln_apply(s-1) is emitted at the top of
    iteration s (attention of s covers its PE-bcast/ACT/DVE chain); the LN
    row-stat chain for strip s is emitted inside FFN2 of ffn(s-1) where the
    ACT engine is idle, keeping its 2 act-table loads off the gelu stream.
"""

import numpy as np
import ml_dtypes

import concourse.bass as bass
from concourse import bacc, tile, mybir
from concourse.bass_utils import run_bass_kernel_spmd

BF16 = ml_dtypes.bfloat16
F8 = ml_dtypes.float8_e4m3
DT_BF = mybir.dt.bfloat16
DT_F8 = mybir.dt.float8e4
DT_F32 = mybir.dt.float32
AF = mybir.ActivationFunctionType
ALU = mybir.AluOpType
DR = mybir.MatmulPerfMode.DoubleRow

B_FULL, E, H = 16384, 768, 8
F = 4 * E  # 3072
N_CORES = 8
BS = B_FULL // N_CORES  # 2048
EPS = 1e-5
P = 128
KE = E // P  # 6
KF = F // P  # 24
SW = 256.0   # fp8 weight scale
SX = 16.0    # fp8 activation scale
STRIPS = (256, 512, 512, 512, 256)
MS = 512     # max strip width (tile allocation size)


def build(bs=BS, strips=STRIPS, use_gelu=True, num_devices=N_CORES,
          sim_safe=False):
    """Emit the per-core Bass program (SPMD: same program on every core)."""
    assert sum(strips) == bs
    nstrip = len(strips)
    offs = [sum(strips[:i]) for i in range(nstrip)]

    nc = bacc.Bacc(
        "TRN2", target_bir_lowering=False, debug=False, num_devices=num_devices
    )

    # ---- DRAM I/O ----
    d_img = nc.dram_tensor("imageT", [E, bs], DT_BF, kind="ExternalInput")
    d_txt = nc.dram_tensor("textT", [E, bs], DT_BF, kind="ExternalInput")
    d_watt = {
        "it": nc.dram_tensor("watt_it", [E, E], DT_BF, kind="ExternalInput"),
        "ti": nc.dram_tensor("watt_ti", [E, E], DT_BF, kind="ExternalInput"),
    }
    d_wfp = nc.dram_tensor("wfp", [2 * E, E], DT_BF, kind="ExternalInput")
    d_w1 = {
        p: nc.dram_tensor(f"w1_{p}", [E, F], DT_F8, kind="ExternalInput")
        for p in ("fi", "ft")
    }
    d_w2 = {
        p: nc.dram_tensor(f"w2_{p}", [F, E], DT_F8, kind="ExternalInput")
        for p in ("fi", "ft")
    }
    bias_specs = {
        "batt_it": KE, "b1_fi": KF,
        "batt_ti": KE, "b1_ft": KF,
        "bfp": KE, "g_fp": KE, "b_fp_ln": KE,
    }
    d_bias = {
        n: nc.dram_tensor(n, [P, k], DT_F32, kind="ExternalInput")
        for n, k in bias_specs.items()
    }
    d_out = nc.dram_tensor("outT", [E, bs], DT_F32, kind="ExternalOutput")

    def dview(d):  # [E|2E, bs] dram -> [p, kt, n] view
        return d.ap().rearrange("(kt p) n -> p kt n", p=P)

    with tile.TileContext(nc) as tc:
        from contextlib import ExitStack

        with ExitStack() as ctx:
            const = ctx.enter_context(tc.tile_pool(name="const", bufs=1))
            pin = ctx.enter_context(tc.tile_pool(name="pin", bufs=2))
            pwork = ctx.enter_context(tc.tile_pool(name="pwork", bufs=2))
            ph = ctx.enter_context(tc.tile_pool(name="ph", bufs=1))
            prow = ctx.enter_context(tc.tile_pool(name="prow", bufs=1))
            pst = ctx.enter_context(tc.tile_pool(name="pst", bufs=3))
            pps = ctx.enter_context(
                tc.tile_pool(name="pps", bufs=2, space=bass.MemorySpace.PSUM)
            )
            pdram = ctx.enter_context(
                tc.tile_pool(name="pdram", bufs=1, space=bass.MemorySpace.DRAM)
            )

            # ---- constants needed for SP1 start (small, DMA'd first) ----
            ones_sb = const.tile([P, P], DT_BF)
            nc.vector.memset(ones_sb[:], 1.0)
            eps_sb = const.tile([1, 1], DT_F32)
            nc.vector.memset(eps_sb[:], EPS)
            watt_sb = {
                pfx: const.tile(
                    [P, KE, E], DT_BF, tag=f"watt_{pfx}", name=f"watt_{pfx}"
                )
                for pfx in ("it", "ti")
            }
            for k in range(KE):
                nc.sync.dma_start(
                    watt_sb["it"][:, k, :], dview(d_watt["it"])[:, k, :]
                )
            bias_sb = {}
            for n, k in bias_specs.items():
                t = const.tile([P, k], DT_F32, tag=f"bias_{n}")
                nc.sync.dma_start(t[:], d_bias[n].ap())
                bias_sb[n] = t
            wfp_sb = const.tile([P, 2 * KE, E], DT_BF)
            # (watt_ti / wfp DMAs are emitted at SP1-strip0 / SP2-strip0)

            # ---- internal DRAM trunk: per-strip tiles for fine-grained deps --
            d_img2 = [
                pdram.tile([P, KE, strips[s]], DT_BF, tag=f"img2_{s}",
                           name=f"img2_{s}")
                for s in range(nstrip)
            ]
            d_txt2 = [
                pdram.tile([P, KE, strips[s]], DT_BF, tag=f"txt2_{s}",
                           name=f"txt2_{s}")
                for s in range(nstrip)
            ]

            # ---------- helpers (sz = current strip width) ----------
            def load_strip_ext(dsrc, s, tag):
                sl = slice(offs[s], offs[s] + strips[s])
                t = pin.tile([P, KE, MS], DT_BF, tag=tag, name=f"in_{tag}")
                nc.sync.dma_start(t[:, :, :strips[s]], dsrc[:, :, sl])
                return t

            def load_strip_trunk(dtile, s, tag):
                t = pin.tile([P, KE, MS], DT_BF, tag=tag, name=f"in_{tag}")
                nc.sync.dma_start(t[:, :, :strips[s]], dtile[:])
                return t

            def dense_att(sz, rhs_t, resid_t, w_sb, b_sb):
                """r[m] = (x @ Wc.T).T[m] + b[m] + resid[m]  (bf16 out)."""
                r = pwork.tile([P, KE, MS], DT_BF, tag="r1", name="r1")
                for m in range(KE):
                    ps = pps.tile([P, MS], DT_F32, tag="mm", bufs=4, name="ps")
                    for k in range(KE):
                        nc.tensor.matmul(
                            ps[:, :sz],
                            w_sb[:, k, m * P:(m + 1) * P],
                            rhs_t[:, k, :sz],
                            start=(k == 0),
                            stop=(k == KE - 1),
                        )
                    nc.vector.scalar_tensor_tensor(
                        r[:, m, :sz], ps[:, :sz], b_sb[:, m:m + 1],
                        resid_t[:, m, :sz], ALU.add, ALU.add,
                    )
                return r

            def ln_presum(sz, r):
                """DVE feature pre-sums of r and r^2 -> [P,sz] bf16 pair."""
                s = pwork.tile([P, MS], DT_BF, tag="s", name="s")
                nc.vector.tensor_tensor(
                    s[:, :sz], r[:, 0, :sz], r[:, 1, :sz], ALU.add)
                for k in range(2, KE):
                    nc.vector.tensor_tensor(
                        s[:, :sz], s[:, :sz], r[:, k, :sz], ALU.add)
                sq = pwork.tile([P, MS], DT_BF, tag="sq", name="sq")
                tmp = pwork.tile([P, MS], DT_BF, tag="sqtmp", name="sqtmp")
                nc.vector.tensor_tensor(
                    sq[:, :sz], r[:, 0, :sz], r[:, 0, :sz], ALU.mult)
                for k in range(1, KE):
                    nc.vector.tensor_tensor(
                        tmp[:, :sz], r[:, k, :sz], r[:, k, :sz], ALU.mult)
                    nc.vector.tensor_tensor(
                        sq[:, :sz], sq[:, :sz], tmp[:, :sz], ALU.add)
                return s, sq

            def ln_redrows(sz, ssq):
                """PE partition-reduce + row-stat chain -> (mean, rstd) rows."""
                s, sq = ssq
                red0 = pps.tile([1, MS], DT_F32, tag="hps", bufs=2, name="red0")
                red1 = pps.tile([1, MS], DT_F32, tag="ops", bufs=2, name="red1")
                nc.tensor.matmul(red0[:, :sz], ones_sb[:, 0:1], s[:, :sz],
                                 start=True, stop=True)
                nc.tensor.matmul(red1[:, :sz], ones_sb[:, 0:1], sq[:, :sz],
                                 start=True, stop=True)
                mean_bf = prow.tile([1, MS], DT_BF, tag="mean", name="mean")
                nc.scalar.activation(mean_bf[:, :sz], red0[:, :sz], AF.Copy,
                                     scale=1.0 / E)
                msq = prow.tile([1, MS], DT_F32, tag="msq", name="msq")
                nc.vector.tensor_tensor(msq[:, :sz], mean_bf[:, :sz],
                                        mean_bf[:, :sz], ALU.mult)
                var = prow.tile([1, MS], DT_F32, tag="var", name="var")
                nc.vector.scalar_tensor_tensor(
                    var[:, :sz], red1[:, :sz], 1.0 / E, msq[:, :sz],
                    ALU.mult, ALU.subtract
                )
                rstd_bf = prow.tile([1, MS], DT_BF, tag="rstdbf", name="rstdbf")
                if sim_safe:
                    std = prow.tile([1, MS], DT_F32, tag="std", name="std")
                    nc.scalar.activation(std[:, :sz], var[:, :sz], AF.Sqrt,
                                         bias=eps_sb[0:1, 0:1])
                    rstd = prow.tile([1, MS], DT_F32, tag="rstd", name="rstd")
                    nc.vector.reciprocal(rstd[:, :sz], std[:, :sz])
                    nc.vector.tensor_copy(rstd_bf[:, :sz], rstd[:, :sz])
                else:
                    nc.scalar.activation(
                        rstd_bf[:, :sz], var[:, :sz], AF.Abs_reciprocal_sqrt,
                        bias=eps_sb[0:1, 0:1],
                    )
                return mean_bf, rstd_bf

            def ln_bcast(sz, rows):
                """PE K=1 broadcast of stats + ACT copy to SBUF bf16 pair."""
                mean_bf, rstd_bf = rows
                mb = pps.tile([P, MS], DT_F32, tag="hps", bufs=2, name="mb")
                nc.tensor.matmul(mb[:, :sz], ones_sb[0:1, :], mean_bf[:, :sz],
                                 start=True, stop=True)
                rb = pps.tile([P, MS], DT_F32, tag="ops", bufs=2, name="rb")
                nc.tensor.matmul(rb[:, :sz], ones_sb[0:1, :], rstd_bf[:, :sz],
                                 start=True, stop=True)
                mbS = pwork.tile([P, MS], DT_BF, tag="mbS", name="mbS")
                nc.scalar.activation(mbS[:, :sz], mb[:, :sz], AF.Copy)
                rbS = pwork.tile([P, MS], DT_BF, tag="rbS", name="rbS")
                nc.scalar.activation(rbS[:, :sz], rb[:, :sz], AF.Copy)
                return mbS, rbS

            def ln_apply(sz, r, rows, want_fp8):
                """t = (r - mean)*rstd (bf16); t8 = fp8(SX*t) via ACT."""
                mbS, rbS = ln_bcast(sz, rows)
                t = pwork.tile([P, KE, MS], DT_BF, tag="t", name="t")
                t8 = (
                    pwork.tile([P, KE, MS], DT_F8, tag="t8", name="t8")
                    if want_fp8 else None
                )
                tb = pwork.tile([P, MS], DT_BF, tag="tb", name="tb")
                for k in range(KE):
                    nc.vector.tensor_tensor(
                        tb[:, :sz], r[:, k, :sz], mbS[:, :sz], ALU.subtract)
                    nc.vector.tensor_tensor(
                        t[:, k, :sz], tb[:, :sz], rbS[:, :sz], ALU.mult)
                    if want_fp8:
                        # ACT engine: idle at iter top (gelu batch not yet
                        # unblocked), and Copy needs no act-table switch.
                        nc.scalar.activation(
                            t8[:, k, :sz], t[:, k, :sz], AF.Copy, scale=SX)
                return t, t8

            def ffn(sz, t, t8, w1, w2, b1_sb, dtile, mid_hook=None):
                """dtile[:, m, :] = t + (gelu-ffn in fp8 DoubleRow)/SW.

                mid_hook (if given) fires after FFN2 m==1: the ACT engine is
                idle during FFN2 (st' epilogues are DVE), so the LN row-stat
                chain + its ACT table switches land off the gelu stream."""
                h8 = ph.tile([P, KF, MS], DT_F8, tag="h", name="h")
                for m in range(KF):
                    hps = pps.tile([P, MS], DT_F32, tag="hps", bufs=2, name="hps")
                    for j in range(KE // 2):
                        nc.tensor.matmul(
                            hps[:, :sz],
                            w1[:, 2 * j:2 * j + 2, m * P:(m + 1) * P],
                            t8[:, 2 * j:2 * j + 2, :sz],
                            start=(j == 0), stop=(j == KE // 2 - 1),
                            perf_mode=DR,
                        )
                    nc.scalar.activation(
                        h8[:, m, :sz], hps[:, :sz],
                        AF.Gelu if use_gelu else AF.Identity,
                        bias=b1_sb[:, m:m + 1], scale=1.0 / (SX * SW),
                    )
                for m in range(KE):
                    ops = pps.tile([P, MS], DT_F32, tag="ops", bufs=2, name="ops")
                    for j in range(KF // 2):
                        nc.tensor.matmul(
                            ops[:, :sz],
                            w2[:, 2 * j:2 * j + 2, m * P:(m + 1) * P],
                            h8[:, 2 * j:2 * j + 2, :sz],
                            start=(j == 0), stop=(j == KF // 2 - 1),
                            perf_mode=DR,
                        )
                    st = pst.tile([P, MS], DT_BF, tag="stg", name="stg")
                    nc.vector.scalar_tensor_tensor(
                        st[:, :sz], ops[:, :sz], 1.0 / SW, t[:, m, :sz],
                        ALU.mult, ALU.add,
                    )
                    nc.sync.dma_start(dtile[:, m, :], st[:, :sz])
                    if m == 1 and mid_hook is not None:
                        mid_hook()

            def superphase(pfx, rhs_src, res_view, watt, b_att, w1d, w2d, b1,
                           dout):
                """rhs_src: callable s -> rhs strip tile; res_view: dram view."""
                with tc.tile_pool(name=f"wffn_{pfx}", bufs=1) as wp:
                    w1 = wp.tile([P, KE, F], DT_F8, tag="w1", name="w1")
                    w2 = wp.tile([P, KF, E], DT_F8, tag="w2", name="w2")

                    def load_w():
                        for k in range(KE):
                            nc.sync.dma_start(
                                w1[:, k, :], w1d.ap()[k * P:(k + 1) * P, :]
                            )
                        for k in range(KF):
                            nc.sync.dma_start(
                                w2[:, k, :], w2d.ap()[k * P:(k + 1) * P, :]
                            )

                    pend = None  # (r, rows, strip idx) awaiting apply+ffn
                    for s in range(nstrip):
                        sz = strips[s]
                        rhs_t = rhs_src(s)
                        res_t = load_strip_ext(res_view, s, "res_in")

                        # apply(s-1) at iter top: all of att(s) covers its
                        # PE-bcast + ACT-copy + DVE chain.
                        cur = None
                        if pend is not None:
                            rp, rowsp, sp = pend
                            cur = ln_apply(strips[sp], rp, rowsp, want_fp8=True)

                        r = dense_att(sz, rhs_t, res_t, watt, b_att)
                        ssq = ln_presum(sz, r)
                        if s == 0 and pfx == "fi":
                            load_w()  # after the first strip's work is queued
                            for k in range(KE):
                                nc.sync.dma_start(
                                    watt_sb["ti"][:, k, :],
                                    dview(d_watt["ti"])[:, k, :],
                                )
                        rbox = [None]
                        if pend is not None:
                            _, _, sp = pend
                            ffn(strips[sp], cur[0], cur[1], w1, w2, b1,
                                dout[sp],
                                mid_hook=lambda: rbox.__setitem__(
                                    0, ln_redrows(sz, ssq)))
                        if s == 0 and pfx != "fi":
                            load_w()
                            for k in range(2 * KE):
                                nc.sync.dma_start(
                                    wfp_sb[:, k, :], dview(d_wfp)[:, k, :]
                                )
                        rows = (rbox[0] if rbox[0] is not None
                                else ln_redrows(sz, ssq))
                        pend = (r, rows, s)
                    rp, rowsp, sp = pend
                    t, t8 = ln_apply(strips[sp], rp, rowsp, want_fp8=True)
                    ffn(strips[sp], t, t8, w1, w2, b1, dout[sp])

            # ---- SP1: image branch (kv = text, residual = image) ----
            superphase(
                "fi",
                lambda s: load_strip_ext(dview(d_txt), s, "rhs_in"),
                dview(d_img), watt_sb["it"], bias_sb["batt_it"],
                d_w1["fi"], d_w2["fi"], bias_sb["b1_fi"],
                d_img2,
            )
            # ---- SP2: text branch (kv = img2, residual = text) ----
            superphase(
                "ft",
                lambda s: load_strip_trunk(d_img2[s], s, "rhs_in"),
                dview(d_txt), watt_sb["ti"], bias_sb["batt_ti"],
                d_w1["ft"], d_w2["ft"], bias_sb["b1_ft"],
                d_txt2,
            )

            # ---- SP3: fused projection + LN + gelu ----
            with tc.tile_pool(name="sp3", bufs=3) as p3:
                outv = dview(d_out)

                def fp_finish(szp, rp3, sp, rowsp):
                    slp = slice(offs[sp], offs[sp] + strips[sp])
                    mbS, rbS = ln_bcast(szp, rowsp)
                    for k in range(KE):
                        tb3 = p3.tile([P, MS], DT_BF, tag="tb3", name="tb3")
                        t3 = p3.tile([P, MS], DT_BF, tag="t3", name="t3")
                        nc.vector.tensor_tensor(
                            tb3[:, :szp], rp3[:, k, :szp], mbS[:, :szp],
                            ALU.subtract)
                        nc.vector.tensor_tensor(
                            t3[:, :szp], tb3[:, :szp], rbS[:, :szp], ALU.mult)
                        o = p3.tile([P, MS], DT_F32, tag="of32", name="of32")
                        nc.scalar.activation(
                            o[:, :szp], t3[:, :szp],
                            AF.Gelu if use_gelu else AF.Identity,
                            bias=bias_sb["b_fp_ln"][:, k:k + 1],
                            scale=bias_sb["g_fp"][:, k:k + 1],
                        )
                        nc.sync.dma_start(outv[:, k, slp], o[:, :szp])

                # 1-strip skew with in-loop hooks: red(s-1) after m1,
                # finish(s-1) after m5 -- PE never waits on the LN chain.
                stages = []  # per strip dict: r3, s, ssq, rows
                for s in range(nstrip):
                    sz = strips[s]
                    a_in = load_strip_trunk(d_img2[s], s, "rhs_in")
                    b_in = load_strip_trunk(d_txt2[s], s, "res_in")
                    r3 = pwork.tile([P, KE, MS], DT_BF, tag="r1", name="r3")
                    for m in range(KE):
                        zps = pps.tile([P, MS], DT_F32, tag="mm", bufs=4,
                                       name="zps")
                        for k in range(2 * KE):
                            src = a_in if k < KE else b_in
                            nc.tensor.matmul(
                                zps[:, :sz], wfp_sb[:, k, m * P:(m + 1) * P],
                                src[:, k % KE, :sz],
                                start=(k == 0), stop=(k == 2 * KE - 1),
                            )
                        nc.scalar.activation(
                            r3[:, m, :sz], zps[:, :sz], AF.Identity,
                            bias=bias_sb["bfp"][:, m:m + 1],
                        )
                        if m == 1 and stages and "rows" not in stages[-1]:
                            st1 = stages[-1]
                            st1["rows"] = ln_redrows(strips[st1["s"]],
                                                     st1["ssq"])
                        if m == 5 and stages and not stages[-1].get("done"):
                            st1 = stages[-1]
                            fp_finish(strips[st1["s"]], st1["r3"], st1["s"],
                                      st1["rows"])
                            st1["done"] = True
                    stages.append({"r3": r3, "s": s, "ssq": ln_presum(sz, r3)})
                last = stages[-1]
                last["rows"] = ln_redrows(strips[last["s"]], last["ssq"])
                fp_finish(strips[last["s"]], last["r3"], last["s"],
                          last["rows"])

    nc.compile()
    return nc


# ---------------- host side ----------------

_BUILT = {}


def _get_nc(key):
    if key not in _BUILT:
        _BUILT[key] = build(*key)
    return _BUILT[key]


def _packv(v, ktiles):
    return np.ascontiguousarray(np.asarray(v, np.float32).reshape(ktiles, P).T)


def prep_inputs(inputs, bs=BS, n_cores=N_CORES):
    f32 = np.float32
    g = lambda n: np.asarray(inputs[n], f32)
    g_img, b_img = g("ln_img_g"), g("ln_img_b")
    g_txt, b_txt = g("ln_text_g"), g("ln_text_b")
    c_img = b_img + g("fi_b2")  # constant the img trunk omits
    c_txt = b_txt + g("ft_b2")
    common = {}
    # --- attention (bf16): Wc = Wo@Wv; ti's rows absorb g_img, bias absorbs
    # the img trunk's missing constant c_img.
    wc_it = g("it_Wo") @ g("it_Wv")
    bc_it = g("it_Wo") @ g("it_bv") + g("it_bo")
    common["watt_it"] = np.ascontiguousarray(wc_it.T).astype(BF16)
    common["batt_it"] = _packv(bc_it, KE)
    wc_ti = g("ti_Wo") @ g("ti_Wv")
    bc_ti = g("ti_Wo") @ g("ti_bv") + g("ti_bo") + wc_ti @ c_img
    common["watt_ti"] = np.ascontiguousarray(wc_ti.T * g_img[:, None]).astype(BF16)
    common["batt_ti"] = _packv(bc_ti, KE)
    # --- FFN (fp8): W1 rows absorb g_ln (input is t, not x); W2 columns are
    # divided by g_ln (trunk carries t + ffn/g); biases b1 absorb b_ln@W1.T.
    for p, gl, bl in (("fi", g_img, b_img), ("ft", g_txt, b_txt)):
        w1 = g(f"{p}_W1")  # [F, E]
        w2 = g(f"{p}_W2")  # [E, F]
        common[f"w1_{p}"] = np.ascontiguousarray(
            w1.T * (gl[:, None] * SW)).astype(F8)
        common[f"w2_{p}"] = np.ascontiguousarray(
            w2.T * (SW / gl[None, :])).astype(F8)
        common[f"b1_{p}"] = _packv(g(f"{p}_b1") + w1 @ bl, KF)
    # --- fused projection (bf16): rows absorb [g_img; g_txt]; bias absorbs
    # the trunks' missing constants.
    fpw = g("fp_W")  # [E, 2E]
    g_cat = np.concatenate([g_img, g_txt])
    c_cat = np.concatenate([c_img, c_txt])
    common["wfp"] = np.ascontiguousarray(fpw.T * g_cat[:, None]).astype(BF16)
    common["bfp"] = _packv(g("fp_b") + fpw @ c_cat, KE)
    common["g_fp"] = _packv(g("fp_ln_g"), KE)
    common["b_fp_ln"] = _packv(g("fp_ln_b"), KE)

    imgT = g("image_embed").T.astype(BF16)
    txtT = g("text_embed").T.astype(BF16)
    in_maps = []
    for c in range(n_cores):
        sl = slice(c * bs, (c + 1) * bs)
        m = dict(common)
        m["imageT"] = np.ascontiguousarray(imgT[:, sl])
        m["textT"] = np.ascontiguousarray(txtT[:, sl])
        in_maps.append(m)
    return in_maps


CFG = (BS, STRIPS, True, N_CORES)


def kernel(**inputs):
    nc = _get_nc(CFG)
    in_maps = prep_inputs(inputs)
    res = run_bass_kernel_spmd(nc, in_maps, core_ids=list(range(N_CORES)))
    out = np.concatenate(
        [res.results[c]["outT"] for c in range(N_CORES)], axis=1
    )  # [E, B]
    return np.ascontiguousarray(out.T).astype(np.float32)


# revision 8
# speedup vs baseline: 1.4514x; 1.0164x over previous
"""Trainium2 Bass kernel: CrossAttentionFusion (dense transformer block pair).

Math notes (vs the reference):
  - seq_len-1 cross attention: softmax over a single key is identically 1, so
    mha1(q_in, kv_in) == kv_in @ (Wo@Wv).T + (Wo@bv + bo).  q/k projections are
    dead code; the two projections fuse into ONE 768x768 matmul (host-fused).
  - Transposed layout: activations live as [feature, batch]; matmuls are
    lhsT(=W.T, stationary) x rhs(=x.T, moving) -> out = (x@W.T).T.
    LayerNorm reduces over features (= partitions) with a ones-vector matmul on
    the PE; per-sample stats are broadcast back over partitions with K=1 ones
    matmuls.
  - FFN matmuls run in fp8e4 (e4m3) with MatmulPerfMode.DoubleRow: one PE
    instruction contracts TWO 128-row k-subtiles (2x bf16 FLOP rate).  Weights
    are host-quantized with a 256x scale (so sigma~0.02 values sit in e4m3's
    normal range); activations are quantized by ACT ops writing fp8 directly
    (t8 = Copy(16*t), h8 = Gelu out).  Scales unwind in the epilogues.
  - LayerNorm gain/bias are folded host-side: g into W1 rows / watt_ti rows /
    wfp rows (trunk carries t + ffn/g, i.e. the pre-gain stream); the constant
    c = b_ln + b2 folds into downstream attention/fp biases.  On-chip LN apply
    is only (r - mean)*rstd.
  - Attention + fused projection stay bf16 (their fp8 noise would land
    directly on the output and blow the 2e-2 budget; they are ~1/3 of MACs).
  - Data-parallel over batch: 16384 rows -> 8 cores x 2048.
  - Strips of [256,512,512,512,256]: small edge strips halve the un-hideable
    pipeline fill (first-strip LN chain) and drain (last-strip epilogue).
  - Software pipeline, 1-strip skew: ln_apply(s-1) is emitted at the top of
    iteration s (attention of s covers its PE-bcast/ACT/DVE chain); the LN
    row-stat chain for strip s is emitted inside FFN2 of ffn(s-1) where the
    ACT engine is idle, keeping its 2 act-table loads off the gelu stream.
"""

import numpy as np
import ml_dtypes

import concourse.bass as bass
from concourse import bacc, tile, mybir
from concourse.bass_utils import run_bass_kernel_spmd

BF16 = ml_dtypes.bfloat16
F8 = ml_dtypes.float8_e4m3
DT_BF = mybir.dt.bfloat16
DT_F8 = mybir.dt.float8e4
DT_F32 = mybir.dt.float32
AF = mybir.ActivationFunctionType
ALU = mybir.AluOpType
DR = mybir.MatmulPerfMode.DoubleRow

B_FULL, E, H = 16384, 768, 8
F = 4 * E  # 3072
N_CORES = 8
BS = B_FULL // N_CORES  # 2048
EPS = 1e-5
P = 128
KE = E // P  # 6
KF = F // P  # 24
SW = 256.0   # fp8 weight scale
SX = 16.0    # fp8 activation scale
STRIPS = (256, 512, 512, 512, 256)
MS = 512     # max strip width (tile allocation size)


def build(bs=BS, strips=STRIPS, use_gelu=True, num_devices=N_CORES,
          sim_safe=False):
    """Emit the per-core Bass program (SPMD: same program on every core)."""
    assert sum(strips) == bs
    nstrip = len(strips)
    offs = [sum(strips[:i]) for i in range(nstrip)]

    nc = bacc.Bacc(
        "TRN2", target_bir_lowering=False, debug=False, num_devices=num_devices
    )

    # ---- DRAM I/O ----
    d_img = nc.dram_tensor("imageT", [E, bs], DT_BF, kind="ExternalInput")
    d_txt = nc.dram_tensor("textT", [E, bs], DT_BF, kind="ExternalInput")
    d_watt = {
        "it": nc.dram_tensor("watt_it", [E, E], DT_BF, kind="ExternalInput"),
        "ti": nc.dram_tensor("watt_ti", [E, E], DT_BF, kind="ExternalInput"),
    }
    d_wfp = nc.dram_tensor("wfp", [2 * E, E], DT_BF, kind="ExternalInput")
    d_w1 = {
        p: nc.dram_tensor(f"w1_{p}", [E, F], DT_F8, kind="ExternalInput")
        for p in ("fi", "ft")
    }
    d_w2 = {
        p: nc.dram_tensor(f"w2_{p}", [F, E], DT_F8, kind="ExternalInput")
        for p in ("fi", "ft")
    }
    bias_specs = {
        "batt_it": KE, "b1_fi": KF,
        "batt_ti": KE, "b1_ft": KF,
        "bfp": KE, "g_fp": KE, "b_fp_ln": KE,
    }
    d_bias = {
        n: nc.dram_tensor(n, [P, k], DT_F32, kind="ExternalInput")
        for n, k in bias_specs.items()
    }
    d_out = nc.dram_tensor("outT", [E, bs], DT_F32, kind="ExternalOutput")

    def dview(d):  # [E|2E, bs] dram -> [p, kt, n] view
        return d.ap().rearrange("(kt p) n -> p kt n", p=P)

    with tile.TileContext(nc) as tc:
        from contextlib import ExitStack

        with ExitStack() as ctx:
            const = ctx.enter_context(tc.tile_pool(name="const", bufs=1))
            pin = ctx.enter_context(tc.tile_pool(name="pin", bufs=2))
            pwork = ctx.enter_context(tc.tile_pool(name="pwork", bufs=2))
            ph = ctx.enter_context(tc.tile_pool(name="ph", bufs=1))
            prow = ctx.enter_context(tc.tile_pool(name="prow", bufs=1))
            pst = ctx.enter_context(tc.tile_pool(name="pst", bufs=3))
            pps = ctx.enter_context(
                tc.tile_pool(name="pps", bufs=2, space=bass.MemorySpace.PSUM)
            )
            pdram = ctx.enter_context(
                tc.tile_pool(name="pdram", bufs=1, space=bass.MemorySpace.DRAM)
            )

            # ---- constants needed for SP1 start (small, DMA'd first) ----
            ones_sb = const.tile([P, P], DT_BF)
            nc.vector.memset(ones_sb[:], 1.0)
            eps_sb = const.tile([1, 1], DT_F32)
            nc.vector.memset(eps_sb[:], EPS)
            watt_sb = {
                pfx: const.tile(
                    [P, KE, E], DT_BF, tag=f"watt_{pfx}", name=f"watt_{pfx}"
                )
                for pfx in ("it", "ti")
            }
            for k in range(KE):
                nc.sync.dma_start(
                    watt_sb["it"][:, k, :], dview(d_watt["it"])[:, k, :]
                )
            bias_sb = {}
            for n, k in bias_specs.items():
                t = const.tile([P, k], DT_F32, tag=f"bias_{n}")
                nc.sync.dma_start(t[:], d_bias[n].ap())
                bias_sb[n] = t
            wfp_sb = const.tile([P, 2 * KE, E], DT_BF)
            # (watt_ti / wfp DMAs are emitted at SP1-strip0 / SP2-strip0)

            # ---- internal DRAM trunk: per-strip tiles for fine-grained deps --
            d_img2 = [
                pdram.tile([P, KE, strips[s]], DT_BF, tag=f"img2_{s}",
                           name=f"img2_{s}")
                for s in range(nstrip)
            ]
            d_txt2 = [
                pdram.tile([P, KE, strips[s]], DT_BF, tag=f"txt2_{s}",
                           name=f"txt2_{s}")
                for s in range(nstrip)
            ]

            # ---------- helpers (sz = current strip width) ----------
            def load_strip_ext(dsrc, s, tag):
                sl = slice(offs[s], offs[s] + strips[s])
                t = pin.tile([P, KE, MS], DT_BF, tag=tag, name=f"in_{tag}")
                nc.sync.dma_start(t[:, :, :strips[s]], dsrc[:, :, sl])
                return t

            def load_strip_trunk(dtile, s, tag):
                t = pin.tile([P, KE, MS], DT_BF, tag=tag, name=f"in_{tag}")
                nc.sync.dma_start(t[:, :, :strips[s]], dtile[:])
                return t

            def dense_att(sz, rhs_t, resid_t, w_sb, b_sb):
                """r[m] = (x @ Wc.T).T[m] + b[m] + resid[m]  (bf16 out)."""
                r = pwork.tile([P, KE, MS], DT_BF, tag="r1", name="r1")
                for m in range(KE):
                    ps = pps.tile([P, MS], DT_F32, tag="mm", bufs=4, name="ps")
                    for k in range(KE):
                        nc.tensor.matmul(
                            ps[:, :sz],
                            w_sb[:, k, m * P:(m + 1) * P],
                            rhs_t[:, k, :sz],
                            start=(k == 0),
                            stop=(k == KE - 1),
                        )
                    nc.vector.scalar_tensor_tensor(
                        r[:, m, :sz], ps[:, :sz], b_sb[:, m:m + 1],
                        resid_t[:, m, :sz], ALU.add, ALU.add,
                    )
                return r

            def ln_presum(sz, r):
                """DVE feature pre-sums of r and r^2 -> [P,sz] bf16 pair."""
                s = pwork.tile([P, MS], DT_BF, tag="s", name="s")
                nc.vector.tensor_tensor(
                    s[:, :sz], r[:, 0, :sz], r[:, 1, :sz], ALU.add)
                for k in range(2, KE):
                    nc.vector.tensor_tensor(
                        s[:, :sz], s[:, :sz], r[:, k, :sz], ALU.add)
                sq = pwork.tile([P, MS], DT_BF, tag="sq", name="sq")
                tmp = pwork.tile([P, MS], DT_BF, tag="sqtmp", name="sqtmp")
                nc.vector.tensor_tensor(
                    sq[:, :sz], r[:, 0, :sz], r[:, 0, :sz], ALU.mult)
                for k in range(1, KE):
                    nc.vector.tensor_tensor(
                        tmp[:, :sz], r[:, k, :sz], r[:, k, :sz], ALU.mult)
                    nc.vector.tensor_tensor(
                        sq[:, :sz], sq[:, :sz], tmp[:, :sz], ALU.add)
                return s, sq

            def ln_redrows(sz, ssq):
                """PE partition-reduce + row-stat chain -> (mean, rstd) rows."""
                s, sq = ssq
                red0 = pps.tile([1, MS], DT_F32, tag="hps", bufs=2, name="red0")
                red1 = pps.tile([1, MS], DT_F32, tag="ops", bufs=2, name="red1")
                nc.tensor.matmul(red0[:, :sz], ones_sb[:, 0:1], s[:, :sz],
                                 start=True, stop=True)
                nc.tensor.matmul(red1[:, :sz], ones_sb[:, 0:1], sq[:, :sz],
                                 start=True, stop=True)
                mean_bf = prow.tile([1, MS], DT_BF, tag="mean", name="mean")
                nc.scalar.activation(mean_bf[:, :sz], red0[:, :sz], AF.Copy,
                                     scale=1.0 / E)
                msq = prow.tile([1, MS], DT_F32, tag="msq", name="msq")
                nc.vector.tensor_tensor(msq[:, :sz], mean_bf[:, :sz],
                                        mean_bf[:, :sz], ALU.mult)
                var = prow.tile([1, MS], DT_F32, tag="var", name="var")
                nc.vector.scalar_tensor_tensor(
                    var[:, :sz], red1[:, :sz], 1.0 / E, msq[:, :sz],
                    ALU.mult, ALU.subtract
                )
                rstd_bf = prow.tile([1, MS], DT_BF, tag="rstdbf", name="rstdbf")
                if sim_safe:
                    std = prow.tile([1, MS], DT_F32, tag="std", name="std")
                    nc.scalar.activation(std[:, :sz], var[:, :sz], AF.Sqrt,
                                         bias=eps_sb[0:1, 0:1])
                    rstd = prow.tile([1, MS], DT_F32, tag="rstd", name="rstd")
                    nc.vector.reciprocal(rstd[:, :sz], std[:, :sz])
                    nc.vector.tensor_copy(rstd_bf[:, :sz], rstd[:, :sz])
                else:
                    nc.scalar.activation(
                        rstd_bf[:, :sz], var[:, :sz], AF.Abs_reciprocal_sqrt,
                        bias=eps_sb[0:1, 0:1],
                    )
                return mean_bf, rstd_bf

            def ln_bcast(sz, rows):
                """PE K=1 broadcast of stats + ACT copy to SBUF bf16 pair."""
                mean_bf, rstd_bf = rows
                mb = pps.tile([P, MS], DT_F32, tag="hps", bufs=2, name="mb")
                nc.tensor.matmul(mb[:, :sz], ones_sb[0:1, :], mean_bf[:, :sz],
                                 start=True, stop=True)
                rb = pps.tile([P, MS], DT_F32, tag="ops", bufs=2, name="rb")
                nc.tensor.matmul(rb[:, :sz], ones_sb[0:1, :], rstd_bf[:, :sz],
                                 start=True, stop=True)
                mbS = pwork.tile([P, MS], DT_BF, tag="mbS", name="mbS")
                nc.scalar.activation(mbS[:, :sz], mb[:, :sz], AF.Copy)
                rbS = pwork.tile([P, MS], DT_BF, tag="rbS", name="rbS")
                nc.scalar.activation(rbS[:, :sz], rb[:, :sz], AF.Copy)
                return mbS, rbS

            def ln_apply(sz, r, rows, want_fp8):
                """t = (r - mean)*rstd (bf16); t8 = fp8(SX*t) via ACT."""
                mbS, rbS = ln_bcast(sz, rows)
                t = pwork.tile([P, KE, MS], DT_BF, tag="t", name="t")
                t8 = (
                    pwork.tile([P, KE, MS], DT_F8, tag="t8", name="t8")
                    if want_fp8 else None
                )
                tb = pwork.tile([P, MS], DT_BF, tag="tb", name="tb")
                for k in range(KE):
                    nc.vector.tensor_tensor(
                        tb[:, :sz], r[:, k, :sz], mbS[:, :sz], ALU.subtract)
                    nc.vector.tensor_tensor(
                        t[:, k, :sz], tb[:, :sz], rbS[:, :sz], ALU.mult)
                    if want_fp8:
                        # ACT engine: idle at iter top (gelu batch not yet
                        # unblocked), and Copy needs no act-table switch.
                        nc.scalar.activation(
                            t8[:, k, :sz], t[:, k, :sz], AF.Copy, scale=SX)
                return t, t8

            def ffn(sz, t, t8, w1, w2, b1_sb, dtile, mid_hook=None):
                """dtile[:, m, :] = t + (gelu-ffn in fp8 DoubleRow)/SW.

                mid_hook (if given) fires after FFN2 m==1: the ACT engine is
                idle during FFN2 (st' epilogues are DVE), so the LN row-stat
                chain + its ACT table switches land off the gelu stream."""
                h8 = ph.tile([P, KF, MS], DT_F8, tag="h", name="h")
                for m in range(KF):
                    hps = pps.tile([P, MS], DT_F32, tag="hps", bufs=2, name="hps")
                    for j in range(KE // 2):
                        nc.tensor.matmul(
                            hps[:, :sz],
                            w1[:, 2 * j:2 * j + 2, m * P:(m + 1) * P],
                            t8[:, 2 * j:2 * j + 2, :sz],
                            start=(j == 0), stop=(j == KE // 2 - 1),
                            perf_mode=DR,
                        )
                    nc.scalar.activation(
                        h8[:, m, :sz], hps[:, :sz],
                        AF.Gelu if use_gelu else AF.Identity,
                        bias=b1_sb[:, m:m + 1], scale=1.0 / (SX * SW),
                    )
                for m in range(KE):
                    ops = pps.tile([P, MS], DT_F32, tag="ops", bufs=2, name="ops")
                    for j in range(KF // 2):
                        nc.tensor.matmul(
                            ops[:, :sz],
                            w2[:, 2 * j:2 * j + 2, m * P:(m + 1) * P],
                            h8[:, 2 * j:2 * j + 2, :sz],
                            start=(j == 0), stop=(j == KF // 2 - 1),
                            perf_mode=DR,
                        )
                    st = pst.tile([P, MS], DT_BF, tag="stg", name="stg")
                    nc.vector.scalar_tensor_tensor(
                        st[:, :sz], ops[:, :sz], 1.0 / SW, t[:, m, :sz],
                        ALU.mult, ALU.add,
                    )
                    nc.sync.dma_start(dtile[:, m, :], st[:, :sz])
                    if m == 1 and mid_hook is not None:
                        mid_hook()

            def superphase(pfx, rhs_src, res_view, watt, b_att, w1d, w2d, b1,
                           dout):
                """rhs_src: callable s -> rhs strip tile; res_view: dram view."""
                with tc.tile_pool(name=f"wffn_{pfx}", bufs=1) as wp:
                    w1 = wp.tile([P, KE, F], DT_F8, tag="w1", name="w1")
                    w2 = wp.tile([P, KF, E], DT_F8, tag="w2", name="w2")

                    def load_w():
                        for k in range(KE):
                            nc.sync.dma_start(
                                w1[:, k, :], w1d.ap()[k * P:(k + 1) * P, :]
                            )
                        for k in range(KF):
                            nc.sync.dma_start(
                                w2[:, k, :], w2d.ap()[k * P:(k + 1) * P, :]
                            )

                    pend = None  # (r, rows, strip idx) awaiting apply+ffn
                    loads = {0: (rhs_src(0), load_strip_ext(res_view, 0, "res_in"))}
                    for s in range(nstrip):
                        sz = strips[s]
                        # Prefetch strip s+1's inputs NOW: emitted ahead of
                        # ffn(s-1)'s trunk writes so the sync queue has them
                        # in flight before att(s+1) needs them.
                        if s + 1 < nstrip:
                            loads[s + 1] = (rhs_src(s + 1),
                                            load_strip_ext(res_view, s + 1,
                                                           "res_in"))
                        rhs_t, res_t = loads.pop(s)

                        # apply(s-1) at iter top: all of att(s) covers its
                        # PE-bcast + ACT-copy + DVE chain.
                        cur = None
                        if pend is not None:
                            rp, rowsp, sp = pend
                            cur = ln_apply(strips[sp], rp, rowsp, want_fp8=True)

                        r = dense_att(sz, rhs_t, res_t, watt, b_att)
                        ssq = ln_presum(sz, r)
                        if s == 0 and pfx == "fi":
                            load_w()  # after the first strip's work is queued
                            for k in range(KE):
                                nc.sync.dma_start(
                                    watt_sb["ti"][:, k, :],
                                    dview(d_watt["ti"])[:, k, :],
                                )
                        rbox = [None]
                        if pend is not None:
                            _, _, sp = pend
                            ffn(strips[sp], cur[0], cur[1], w1, w2, b1,
                                dout[sp],
                                mid_hook=lambda: rbox.__setitem__(
                                    0, ln_redrows(sz, ssq)))
                        if s == 0 and pfx != "fi":
                            load_w()
                            for k in range(2 * KE):
                                nc.sync.dma_start(
                                    wfp_sb[:, k, :], dview(d_wfp)[:, k, :]
                                )
                        rows = (rbox[0] if rbox[0] is not None
                                else ln_redrows(sz, ssq))
                        pend = (r, rows, s)
                    rp, rowsp, sp = pend
                    t, t8 = ln_apply(strips[sp], rp, rowsp, want_fp8=True)
                    ffn(strips[sp], t, t8, w1, w2, b1, dout[sp])

            # ---- SP1: image branch (kv = text, residual = image) ----
            superphase(
                "fi",
                lambda s: load_strip_ext(dview(d_txt), s, "rhs_in"),
                dview(d_img), watt_sb["it"], bias_sb["batt_it"],
                d_w1["fi"], d_w2["fi"], bias_sb["b1_fi"],
                d_img2,
            )
            # ---- SP2: text branch (kv = img2, residual = text) ----
            superphase(
                "ft",
                lambda s: load_strip_trunk(d_img2[s], s, "rhs_in"),
                dview(d_txt), watt_sb["ti"], bias_sb["batt_ti"],
                d_w1["ft"], d_w2["ft"], bias_sb["b1_ft"],
                d_txt2,
            )

            # ---- SP3: fused projection + LN + gelu ----
            with tc.tile_pool(name="sp3", bufs=3) as p3:
                outv = dview(d_out)

                def fp_finish(szp, rp3, sp, rowsp):
                    slp = slice(offs[sp], offs[sp] + strips[sp])
                    mbS, rbS = ln_bcast(szp, rowsp)
                    for k in range(KE):
                        tb3 = p3.tile([P, MS], DT_BF, tag="tb3", name="tb3")
                        t3 = p3.tile([P, MS], DT_BF, tag="t3", name="t3")
                        nc.vector.tensor_tensor(
                            tb3[:, :szp], rp3[:, k, :szp], mbS[:, :szp],
                            ALU.subtract)
                        nc.vector.tensor_tensor(
                            t3[:, :szp], tb3[:, :szp], rbS[:, :szp], ALU.mult)
                        o = p3.tile([P, MS], DT_F32, tag="of32", name="of32")
                        nc.scalar.activation(
                            o[:, :szp], t3[:, :szp],
                            AF.Gelu if use_gelu else AF.Identity,
                            bias=bias_sb["b_fp_ln"][:, k:k + 1],
                            scale=bias_sb["g_fp"][:, k:k + 1],
                        )
                        nc.sync.dma_start(outv[:, k, slp], o[:, :szp])

                # 1-strip skew with in-loop hooks: red(s-1) after m1,
                # finish(s-1) after m5 -- PE never waits on the LN chain.
                stages = []  # per strip dict: r3, s, ssq, rows
                loads3 = {0: (load_strip_trunk(d_img2[0], 0, "rhs_in"),
                              load_strip_trunk(d_txt2[0], 0, "res_in"))}
                for s in range(nstrip):
                    sz = strips[s]
                    if s + 1 < nstrip:
                        loads3[s + 1] = (
                            load_strip_trunk(d_img2[s + 1], s + 1, "rhs_in"),
                            load_strip_trunk(d_txt2[s + 1], s + 1, "res_in"))
                    a_in, b_in = loads3.pop(s)
                    r3 = pwork.tile([P, KE, MS], DT_BF, tag="r1", name="r3")
                    for m in range(KE):
                        zps = pps.tile([P, MS], DT_F32, tag="mm", bufs=4,
                                       name="zps")
                        for k in range(2 * KE):
                            src = a_in if k < KE else b_in
                            nc.tensor.matmul(
                                zps[:, :sz], wfp_sb[:, k, m * P:(m + 1) * P],
                                src[:, k % KE, :sz],
                                start=(k == 0), stop=(k == 2 * KE - 1),
                            )
                        nc.scalar.activation(
                            r3[:, m, :sz], zps[:, :sz], AF.Identity,
                            bias=bias_sb["bfp"][:, m:m + 1],
                        )
                        if m == 1 and stages and "rows" not in stages[-1]:
                            st1 = stages[-1]
                            st1["rows"] = ln_redrows(strips[st1["s"]],
                                                     st1["ssq"])
                        if m == 5 and stages and not stages[-1].get("done"):
                            st1 = stages[-1]
                            fp_finish(strips[st1["s"]], st1["r3"], st1["s"],
                                      st1["rows"])
                            st1["done"] = True
                    stages.append({"r3": r3, "s": s, "ssq": ln_presum(sz, r3)})
                last = stages[-1]
                last["rows"] = ln_redrows(strips[last["s"]], last["ssq"])
                fp_finish(strips[last["s"]], last["r3"], last["s"],
                          last["rows"])

    nc.compile()
    return nc


# ---------------- host side ----------------

_BUILT = {}


def _get_nc(key):
    if key not in _BUILT:
        _BUILT[key] = build(*key)
    return _BUILT[key]


def _packv(v, ktiles):
    return np.ascontiguousarray(np.asarray(v, np.float32).reshape(ktiles, P).T)


def prep_inputs(inputs, bs=BS, n_cores=N_CORES):
    f32 = np.float32
    g = lambda n: np.asarray(inputs[n], f32)
    g_img, b_img = g("ln_img_g"), g("ln_img_b")
    g_txt, b_txt = g("ln_text_g"), g("ln_text_b")
    c_img = b_img + g("fi_b2")  # constant the img trunk omits
    c_txt = b_txt + g("ft_b2")
    common = {}
    # --- attention (bf16): Wc = Wo@Wv; ti's rows absorb g_img, bias absorbs
    # the img trunk's missing constant c_img.
    wc_it = g("it_Wo") @ g("it_Wv")
    bc_it = g("it_Wo") @ g("it_bv") + g("it_bo")
    common["watt_it"] = np.ascontiguousarray(wc_it.T).astype(BF16)
    common["batt_it"] = _packv(bc_it, KE)
    wc_ti = g("ti_Wo") @ g("ti_Wv")
    bc_ti = g("ti_Wo") @ g("ti_bv") + g("ti_bo") + wc_ti @ c_img
    common["watt_ti"] = np.ascontiguousarray(wc_ti.T * g_img[:, None]).astype(BF16)
    common["batt_ti"] = _packv(bc_ti, KE)
    # --- FFN (fp8): W1 rows absorb g_ln (input is t, not x); W2 columns are
    # divided by g_ln (trunk carries t + ffn/g); biases b1 absorb b_ln@W1.T.
    for p, gl, bl in (("fi", g_img, b_img), ("ft", g_txt, b_txt)):
        w1 = g(f"{p}_W1")  # [F, E]
        w2 = g(f"{p}_W2")  # [E, F]
        common[f"w1_{p}"] = np.ascontiguousarray(
            w1.T * (gl[:, None] * SW)).astype(F8)
        common[f"w2_{p}"] = np.ascontiguousarray(
            w2.T * (SW / gl[None, :])).astype(F8)
        common[f"b1_{p}"] = _packv(g(f"{p}_b1") + w1 @ bl, KF)
    # --- fused projection (bf16): rows absorb [g_img; g_txt]; bias absorbs
    # the trunks' missing constants.
    fpw = g("fp_W")  # [E, 2E]
    g_cat = np.concatenate([g_img, g_txt])
    c_cat = np.concatenate([c_img, c_txt])
    common["wfp"] = np.ascontiguousarray(fpw.T * g_cat[:, None]).astype(BF16)
    common["bfp"] = _packv(g("fp_b") + fpw @ c_cat, KE)
    common["g_fp"] = _packv(g("fp_ln_g"), KE)
    common["b_fp_ln"] = _packv(g("fp_ln_b"), KE)

    imgT = g("image_embed").T.astype(BF16)
    txtT = g("text_embed").T.astype(BF16)
    in_maps = []
    for c in range(n_cores):
        sl = slice(c * bs, (c + 1) * bs)
        m = dict(common)
        m["imageT"] = np.ascontiguousarray(imgT[:, sl])
        m["textT"] = np.ascontiguousarray(txtT[:, sl])
        in_maps.append(m)
    return in_maps


CFG = (BS, STRIPS, True, N_CORES)


def kernel(**inputs):
    nc = _get_nc(CFG)
    in_maps = prep_inputs(inputs)
    res = run_bass_kernel_spmd(nc, in_maps, core_ids=list(range(N_CORES)))
    out = np.concatenate(
        [res.results[c]["outT"] for c in range(N_CORES)], axis=1
    )  # [E, B]
    return np.ascontiguousarray(out.T).astype(np.float32)


# revision 15
# speedup vs baseline: 1.4890x; 1.0260x over previous
"""Trainium2 Bass kernel: CrossAttentionFusion (dense transformer block pair).

Math notes (vs the reference):
  - seq_len-1 cross attention: softmax over a single key is identically 1, so
    mha1(q_in, kv_in) == kv_in @ (Wo@Wv).T + (Wo@bv + bo).  q/k projections are
    dead code; the two projections fuse into ONE 768x768 matmul (host-fused).
  - Transposed layout: activations live as [feature, batch]; matmuls are
    lhsT(=W.T, stationary) x rhs(=x.T, moving) -> out = (x@W.T).T.
    LayerNorm reduces over features (= partitions) with a ones-vector matmul on
    the PE; per-sample stats are broadcast back over partitions with K=1 ones
    matmuls.
  - FFN matmuls run in fp8e4 (e4m3) with MatmulPerfMode.DoubleRow: one PE
    instruction contracts TWO 128-row k-subtiles (2x bf16 FLOP rate).  Weights
    are host-quantized with a 256x scale (so sigma~0.02 values sit in e4m3's
    normal range); activations are quantized by ACT ops writing fp8 directly
    (t8 = Copy(16*t), h8 = Gelu out).  Scales unwind in the epilogues.
  - LayerNorm gain/bias are folded host-side: g into W1 rows / watt_ti rows /
    wfp rows (trunk carries t + ffn/g, i.e. the pre-gain stream); the constant
    c = b_ln + b2 folds into downstream attention/fp biases.  On-chip LN apply
    is only (r - mean)*rstd.
  - Attention + fused projection stay bf16 (their fp8 noise would land
    directly on the output and blow the 2e-2 budget; they are ~1/3 of MACs).
  - Data-parallel over batch: 16384 rows -> 8 cores x 2048.
  - Strips of [256,512,512,512,256]: small edge strips halve the un-hideable
    pipeline fill (first-strip LN chain) and drain (last-strip epilogue).
  - Software pipeline, 1-strip skew: ln_apply(s-1) is emitted at the top of
    iteration s (attention of s covers its PE-bcast/ACT/DVE chain); the LN
    row-stat chain for strip s is emitted inside FFN2 of ffn(s-1) where the
    ACT engine is idle, keeping its 2 act-table loads off the gelu stream.
"""

import numpy as np
import ml_dtypes

import concourse.bass as bass
from concourse import bacc, tile, mybir
from concourse.bass_utils import run_bass_kernel_spmd

BF16 = ml_dtypes.bfloat16
F8 = ml_dtypes.float8_e4m3
DT_BF = mybir.dt.bfloat16
DT_F8 = mybir.dt.float8e4
DT_F32 = mybir.dt.float32
AF = mybir.ActivationFunctionType
ALU = mybir.AluOpType
DR = mybir.MatmulPerfMode.DoubleRow

B_FULL, E, H = 16384, 768, 8
F = 4 * E  # 3072
N_CORES = 8
BS = B_FULL // N_CORES  # 2048
EPS = 1e-5
P = 128
KE = E // P  # 6
KF = F // P  # 24
SW = 256.0   # fp8 weight scale
SX = 16.0    # fp8 activation scale
STRIPS = (256, 512, 512, 512, 256)
MS = 512     # max strip width (tile allocation size)


def build(bs=BS, strips=STRIPS, use_gelu=True, num_devices=N_CORES,
          sim_safe=False):
    """Emit the per-core Bass program (SPMD: same program on every core)."""
    assert sum(strips) == bs
    nstrip = len(strips)
    offs = [sum(strips[:i]) for i in range(nstrip)]

    nc = bacc.Bacc(
        "TRN2", target_bir_lowering=False, debug=False, num_devices=num_devices
    )

    # ---- DRAM I/O ----
    d_img = nc.dram_tensor("imageT", [E, bs], DT_BF, kind="ExternalInput")
    d_txt = nc.dram_tensor("textT", [E, bs], DT_BF, kind="ExternalInput")
    d_watt = {
        "it": nc.dram_tensor("watt_it", [E, E], DT_BF, kind="ExternalInput"),
        "ti": nc.dram_tensor("watt_ti", [E, E], DT_BF, kind="ExternalInput"),
    }
    d_wfp = nc.dram_tensor("wfp", [2 * E, E], DT_BF, kind="ExternalInput")
    d_w1 = {
        p: nc.dram_tensor(f"w1_{p}", [E, F], DT_F8, kind="ExternalInput")
        for p in ("fi", "ft")
    }
    d_w2 = {
        p: nc.dram_tensor(f"w2_{p}", [F, E], DT_F8, kind="ExternalInput")
        for p in ("fi", "ft")
    }
    bias_specs = {
        "batt_it": KE, "b1_fi": KF,
        "batt_ti": KE, "b1_ft": KF,
        "bfp": KE, "g_fp": KE, "b_fp_ln": KE,
    }
    d_bias = {
        n: nc.dram_tensor(n, [P, k], DT_F32, kind="ExternalInput")
        for n, k in bias_specs.items()
    }
    d_out = nc.dram_tensor("outT", [E, bs], DT_F32, kind="ExternalOutput")

    def dview(d):  # [E|2E, bs] dram -> [p, kt, n] view
        return d.ap().rearrange("(kt p) n -> p kt n", p=P)

    with tile.TileContext(nc) as tc:
        from contextlib import ExitStack

        with ExitStack() as ctx:
            const = ctx.enter_context(tc.tile_pool(name="const", bufs=1))
            pin = ctx.enter_context(tc.tile_pool(name="pin", bufs=2))
            pwork = ctx.enter_context(tc.tile_pool(name="pwork", bufs=2))
            ph = ctx.enter_context(tc.tile_pool(name="ph", bufs=1))
            prow = ctx.enter_context(tc.tile_pool(name="prow", bufs=1))
            pst = ctx.enter_context(tc.tile_pool(name="pst", bufs=3))
            pps = ctx.enter_context(
                tc.tile_pool(name="pps", bufs=2, space=bass.MemorySpace.PSUM)
            )
            pdram = ctx.enter_context(
                tc.tile_pool(name="pdram", bufs=1, space=bass.MemorySpace.DRAM)
            )

            # ---- constants needed for SP1 start (small, DMA'd first) ----
            ones_sb = const.tile([P, P], DT_BF)
            nc.vector.memset(ones_sb[:], 1.0)
            eps_sb = const.tile([1, 1], DT_F32)
            nc.vector.memset(eps_sb[:], EPS)
            watt_sb = {
                pfx: const.tile(
                    [P, KE, E], DT_BF, tag=f"watt_{pfx}", name=f"watt_{pfx}"
                )
                for pfx in ("it", "ti")
            }
            for k in range(KE):
                nc.sync.dma_start(
                    watt_sb["it"][:, k, :], dview(d_watt["it"])[:, k, :]
                )
            bias_sb = {}
            for n, k in bias_specs.items():
                bias_sb[n] = const.tile([P, k], DT_F32, tag=f"bias_{n}",
                                        name=f"bias_{n}")
            wfp_sb = const.tile([P, 2 * KE, E], DT_BF)
            # (watt_ti / wfp DMAs are emitted at SP1-strip0 / SP2-strip0)

            # ---- internal DRAM trunk: per-strip tiles for fine-grained deps --
            d_img2 = [
                pdram.tile([P, KE, strips[s]], DT_BF, tag=f"img2_{s}",
                           name=f"img2_{s}")
                for s in range(nstrip)
            ]
            d_txt2 = [
                pdram.tile([P, KE, strips[s]], DT_BF, tag=f"txt2_{s}",
                           name=f"txt2_{s}")
                for s in range(nstrip)
            ]

            # ---------- helpers (sz = current strip width) ----------
            def load_strip_ext(dsrc, s, tag):
                sl = slice(offs[s], offs[s] + strips[s])
                t = pin.tile([P, KE, MS], DT_BF, tag=tag, name=f"in_{tag}")
                nc.sync.dma_start(t[:, :, :strips[s]], dsrc[:, :, sl])
                return t

            def load_strip_trunk(dtile, s, tag):
                t = pin.tile([P, KE, MS], DT_BF, tag=tag, name=f"in_{tag}")
                nc.sync.dma_start(t[:, :, :strips[s]], dtile[:])
                return t

            def dense_att(sz, rhs_t, resid_t, w_sb, b_sb):
                """r[m] = (x @ Wc.T).T[m] + b[m] + resid[m]  (bf16 out)."""
                r = pwork.tile([P, KE, MS], DT_BF, tag="r1", name="r1")
                for m in range(KE):
                    ps = pps.tile([P, MS], DT_F32, tag="mm", bufs=4, name="ps")
                    for k in range(KE):
                        nc.tensor.matmul(
                            ps[:, :sz],
                            w_sb[:, k, m * P:(m + 1) * P],
                            rhs_t[:, k, :sz],
                            start=(k == 0),
                            stop=(k == KE - 1),
                        )
                    nc.vector.scalar_tensor_tensor(
                        r[:, m, :sz], ps[:, :sz], b_sb[:, m:m + 1],
                        resid_t[:, m, :sz], ALU.add, ALU.add,
                    )
                return r

            def ln_presum(sz, r):
                """DVE feature pre-sums of r and r^2 -> [P,sz] bf16 pair."""
                s = pwork.tile([P, MS], DT_BF, tag="s", name="s")
                nc.vector.tensor_tensor(
                    s[:, :sz], r[:, 0, :sz], r[:, 1, :sz], ALU.add)
                for k in range(2, KE):
                    nc.vector.tensor_tensor(
                        s[:, :sz], s[:, :sz], r[:, k, :sz], ALU.add)
                sq = pwork.tile([P, MS], DT_BF, tag="sq", name="sq")
                tmp = pwork.tile([P, MS], DT_BF, tag="sqtmp", name="sqtmp")
                nc.vector.tensor_tensor(
                    sq[:, :sz], r[:, 0, :sz], r[:, 0, :sz], ALU.mult)
                for k in range(1, KE):
                    nc.vector.tensor_tensor(
                        tmp[:, :sz], r[:, k, :sz], r[:, k, :sz], ALU.mult)
                    nc.vector.tensor_tensor(
                        sq[:, :sz], sq[:, :sz], tmp[:, :sz], ALU.add)
                return s, sq

            def ln_redrows(sz, ssq):
                """PE partition-reduce + row-stat chain -> (mean, rstd) rows."""
                s, sq = ssq
                red0 = pps.tile([1, MS], DT_F32, tag="hps", bufs=2, name="red0")
                red1 = pps.tile([1, MS], DT_F32, tag="ops", bufs=2, name="red1")
                nc.tensor.matmul(red0[:, :sz], ones_sb[:, 0:1], s[:, :sz],
                                 start=True, stop=True)
                nc.tensor.matmul(red1[:, :sz], ones_sb[:, 0:1], sq[:, :sz],
                                 start=True, stop=True)
                mean_bf = prow.tile([1, MS], DT_BF, tag="mean", name="mean")
                nc.scalar.activation(mean_bf[:, :sz], red0[:, :sz], AF.Copy,
                                     scale=1.0 / E)
                msq = prow.tile([1, MS], DT_F32, tag="msq", name="msq")
                nc.vector.tensor_tensor(msq[:, :sz], mean_bf[:, :sz],
                                        mean_bf[:, :sz], ALU.mult)
                var = prow.tile([1, MS], DT_F32, tag="var", name="var")
                nc.vector.scalar_tensor_tensor(
                    var[:, :sz], red1[:, :sz], 1.0 / E, msq[:, :sz],
                    ALU.mult, ALU.subtract
                )
                rstd_bf = prow.tile([1, MS], DT_BF, tag="rstdbf", name="rstdbf")
                if sim_safe:
                    std = prow.tile([1, MS], DT_F32, tag="std", name="std")
                    nc.scalar.activation(std[:, :sz], var[:, :sz], AF.Sqrt,
                                         bias=eps_sb[0:1, 0:1])
                    rstd = prow.tile([1, MS], DT_F32, tag="rstd", name="rstd")
                    nc.vector.reciprocal(rstd[:, :sz], std[:, :sz])
                    nc.vector.tensor_copy(rstd_bf[:, :sz], rstd[:, :sz])
                else:
                    nc.scalar.activation(
                        rstd_bf[:, :sz], var[:, :sz], AF.Abs_reciprocal_sqrt,
                        bias=eps_sb[0:1, 0:1],
                    )
                return mean_bf, rstd_bf

            def ln_bcast(sz, rows):
                """PE K=1 broadcast of stats + ACT copy to SBUF bf16 pair."""
                mean_bf, rstd_bf = rows
                mb = pps.tile([P, MS], DT_F32, tag="hps", bufs=2, name="mb")
                nc.tensor.matmul(mb[:, :sz], ones_sb[0:1, :], mean_bf[:, :sz],
                                 start=True, stop=True)
                rb = pps.tile([P, MS], DT_F32, tag="ops", bufs=2, name="rb")
                nc.tensor.matmul(rb[:, :sz], ones_sb[0:1, :], rstd_bf[:, :sz],
                                 start=True, stop=True)
                mbS = pwork.tile([P, MS], DT_BF, tag="mbS", name="mbS")
                nc.scalar.activation(mbS[:, :sz], mb[:, :sz], AF.Copy)
                rbS = pwork.tile([P, MS], DT_BF, tag="rbS", name="rbS")
                nc.scalar.activation(rbS[:, :sz], rb[:, :sz], AF.Copy)
                return mbS, rbS

            def ln_apply(sz, r, rows, want_fp8):
                """t = (r - mean)*rstd (bf16); t8 = fp8(SX*t) via ACT."""
                mbS, rbS = ln_bcast(sz, rows)
                t = pwork.tile([P, KE, MS], DT_BF, tag="t", name="t")
                t8 = (
                    pwork.tile([P, KE, MS], DT_F8, tag="t8", name="t8")
                    if want_fp8 else None
                )
                tb = pwork.tile([P, MS], DT_BF, tag="tb", name="tb")
                for k in range(KE):
                    nc.vector.tensor_tensor(
                        tb[:, :sz], r[:, k, :sz], mbS[:, :sz], ALU.subtract)
                    nc.vector.tensor_tensor(
                        t[:, k, :sz], tb[:, :sz], rbS[:, :sz], ALU.mult)
                    if want_fp8:
                        # ACT engine: idle at iter top (gelu batch not yet
                        # unblocked), and Copy needs no act-table switch.
                        nc.scalar.activation(
                            t8[:, k, :sz], t[:, k, :sz], AF.Copy, scale=SX)
                return t, t8

            def ffn(sz, t, t8, w1, w2, b1_sb, dtile, hook_rows=None,
                    hook_apply=None):
                """dtile[:, m, :] = t + (gelu-ffn in fp8 DoubleRow)/SW.

                hook_rows fires after FFN2 m==0 (LN row-stat chain for the
                CURRENT strip: ACT idle during FFN2, its table switches land
                off the gelu stream); hook_apply after m==2 (bcast + apply
                chain for the current strip, so t8 completes under the FFN2
                tail and the NEXT ffn starts with zero PE stall)."""
                h8 = ph.tile([P, KF, MS], DT_F8, tag="h", name="h")
                for m in range(KF):
                    hps = pps.tile([P, MS], DT_F32, tag="hps", bufs=2, name="hps")
                    for j in range(KE // 2):
                        nc.tensor.matmul(
                            hps[:, :sz],
                            w1[:, 2 * j:2 * j + 2, m * P:(m + 1) * P],
                            t8[:, 2 * j:2 * j + 2, :sz],
                            start=(j == 0), stop=(j == KE // 2 - 1),
                            perf_mode=DR,
                        )
                    nc.scalar.activation(
                        h8[:, m, :sz], hps[:, :sz],
                        AF.Gelu if use_gelu else AF.Identity,
                        bias=b1_sb[:, m:m + 1], scale=1.0 / (SX * SW),
                    )
                for m in range(KE):
                    ops = pps.tile([P, MS], DT_F32, tag="ops", bufs=2, name="ops")
                    for j in range(KF // 2):
                        nc.tensor.matmul(
                            ops[:, :sz],
                            w2[:, 2 * j:2 * j + 2, m * P:(m + 1) * P],
                            h8[:, 2 * j:2 * j + 2, :sz],
                            start=(j == 0), stop=(j == KF // 2 - 1),
                            perf_mode=DR,
                        )
                    st = pst.tile([P, MS], DT_BF, tag="stg", name="stg")
                    nc.vector.scalar_tensor_tensor(
                        st[:, :sz], ops[:, :sz], 1.0 / SW, t[:, m, :sz],
                        ALU.mult, ALU.add,
                    )
                    nc.sync.dma_start(dtile[:, m, :], st[:, :sz])
                    if m == 0 and hook_rows is not None:
                        hook_rows()
                    if m == 2 and hook_apply is not None:
                        hook_apply()

            def superphase(pfx, rhs_src, res_view, watt, b_att, w1d, w2d, b1,
                           dout, preloads=None):
                """rhs_src: callable s -> rhs strip tile; res_view: dram view."""
                with tc.tile_pool(name=f"wffn_{pfx}", bufs=1) as wp:
                    w1 = wp.tile([P, KE, F], DT_F8, tag="w1", name="w1")
                    w2 = wp.tile([P, KF, E], DT_F8, tag="w2", name="w2")

                    def load_w():
                        for k in range(KE):
                            nc.sync.dma_start(
                                w1[:, k, :], w1d.ap()[k * P:(k + 1) * P, :]
                            )
                        for k in range(KF):
                            nc.sync.dma_start(
                                w2[:, k, :], w2d.ap()[k * P:(k + 1) * P, :]
                            )

                    cur = None       # (t, t8) for strip s-1, input of ffn(s-1)
                    r0rows = None    # rows(0), emitted bare in iter 0
                    r_prev = None
                    loads = {0: preloads if preloads is not None else
                             (rhs_src(0), load_strip_ext(res_view, 0, "res_in"))}
                    for s in range(nstrip):
                        sz = strips[s]
                        # Prefetch strip s+1's inputs NOW: emitted ahead of
                        # ffn(s-1)'s trunk writes so the sync queue has them
                        # in flight before att(s+1) needs them.
                        if s + 1 < nstrip:
                            loads[s + 1] = (rhs_src(s + 1),
                                            load_strip_ext(res_view, s + 1,
                                                           "res_in"))
                        rhs_t, res_t = loads.pop(s)

                        r = dense_att(sz, rhs_t, res_t, watt, b_att)
                        ssq = ln_presum(sz, r)
                        if s == 0 and pfx == "fi":
                            load_w()  # after the first strip's work is queued
                            for k in range(KE):
                                nc.sync.dma_start(
                                    watt_sb["ti"][:, k, :],
                                    dview(d_watt["ti"])[:, k, :],
                                )
                        if s >= 1:
                            if cur is None:  # s == 1: no ffn hosted apply(0)
                                cur = ln_apply(strips[0], r_prev, r0rows,
                                               want_fp8=True)
                            nbox = [None, None]

                            def hook_rows(nbox=nbox, sz=sz, ssq=ssq):
                                nbox[0] = ln_redrows(sz, ssq)

                            def hook_apply(nbox=nbox, sz=sz, r=r):
                                nbox[1] = ln_apply(sz, r, nbox[0],
                                                   want_fp8=True)

                            ffn(strips[s - 1], cur[0], cur[1], w1, w2, b1,
                                dout[s - 1], hook_rows=hook_rows,
                                hook_apply=hook_apply)
                            cur = nbox[1]
                        if s == 0 and pfx != "fi":
                            load_w()
                            for k in range(2 * KE):
                                nc.sync.dma_start(
                                    wfp_sb[:, k, :], dview(d_wfp)[:, k, :]
                                )
                        if s == 0:
                            r0rows = ln_redrows(sz, ssq)
                            r_prev = r
                    ffn(strips[nstrip - 1], cur[0], cur[1], w1, w2, b1,
                        dout[nstrip - 1])

            # ---- SP1: image branch (kv = text, residual = image) ----
            # Strip-0 inputs are DMA'd BEFORE the bias tiles: the sync queue
            # issues in priority order and att(0) only needs watt+rhs+res.
            preload0 = (load_strip_ext(dview(d_txt), 0, "rhs_in"),
                        load_strip_ext(dview(d_img), 0, "res_in"))
            for n in bias_specs:
                nc.sync.dma_start(bias_sb[n][:], d_bias[n].ap())
            superphase(
                "fi",
                lambda s: load_strip_ext(dview(d_txt), s, "rhs_in"),
                dview(d_img), watt_sb["it"], bias_sb["batt_it"],
                d_w1["fi"], d_w2["fi"], bias_sb["b1_fi"],
                d_img2,
                preloads=preload0,
            )
            # ---- SP2: text branch (kv = img2, residual = text) ----
            superphase(
                "ft",
                lambda s: load_strip_trunk(d_img2[s], s, "rhs_in"),
                dview(d_txt), watt_sb["ti"], bias_sb["batt_ti"],
                d_w1["ft"], d_w2["ft"], bias_sb["b1_ft"],
                d_txt2,
            )

            # ---- SP3: fused projection + LN + gelu ----
            with tc.tile_pool(name="sp3", bufs=3) as p3:
                outv = dview(d_out)

                def fp_finish(szp, rp3, sp, rowsp):
                    slp = slice(offs[sp], offs[sp] + strips[sp])
                    mbS, rbS = ln_bcast(szp, rowsp)
                    for k in range(KE):
                        tb3 = p3.tile([P, MS], DT_BF, tag="tb3", name="tb3")
                        t3 = p3.tile([P, MS], DT_BF, tag="t3", name="t3")
                        nc.vector.tensor_tensor(
                            tb3[:, :szp], rp3[:, k, :szp], mbS[:, :szp],
                            ALU.subtract)
                        nc.vector.tensor_tensor(
                            t3[:, :szp], tb3[:, :szp], rbS[:, :szp], ALU.mult)
                        o = p3.tile([P, MS], DT_F32, tag="of32", name="of32")
                        nc.scalar.activation(
                            o[:, :szp], t3[:, :szp],
                            AF.Gelu if use_gelu else AF.Identity,
                            bias=bias_sb["b_fp_ln"][:, k:k + 1],
                            scale=bias_sb["g_fp"][:, k:k + 1],
                        )
                        nc.sync.dma_start(outv[:, k, slp], o[:, :szp])

                # 1-strip skew with in-loop hooks: red(s-1) after m1,
                # finish(s-1) after m5 -- PE never waits on the LN chain.
                stages = []  # per strip dict: r3, s, ssq, rows
                loads3 = {0: (load_strip_trunk(d_img2[0], 0, "rhs_in"),
                              load_strip_trunk(d_txt2[0], 0, "res_in"))}
                for s in range(nstrip):
                    sz = strips[s]
                    if s + 1 < nstrip:
                        loads3[s + 1] = (
                            load_strip_trunk(d_img2[s + 1], s + 1, "rhs_in"),
                            load_strip_trunk(d_txt2[s + 1], s + 1, "res_in"))
                    a_in, b_in = loads3.pop(s)
                    r3 = pwork.tile([P, KE, MS], DT_BF, tag="r1", name="r3")
                    for m in range(KE):
                        zps = pps.tile([P, MS], DT_F32, tag="mm", bufs=4,
                                       name="zps")
                        for k in range(2 * KE):
                            src = a_in if k < KE else b_in
                            nc.tensor.matmul(
                                zps[:, :sz], wfp_sb[:, k, m * P:(m + 1) * P],
                                src[:, k % KE, :sz],
                                start=(k == 0), stop=(k == 2 * KE - 1),
                            )
                        nc.scalar.activation(
                            r3[:, m, :sz], zps[:, :sz], AF.Identity,
                            bias=bias_sb["bfp"][:, m:m + 1],
                        )
                        if m == 1 and stages and "rows" not in stages[-1]:
                            st1 = stages[-1]
                            st1["rows"] = ln_redrows(strips[st1["s"]],
                                                     st1["ssq"])
                        if m == 5 and stages and not stages[-1].get("done"):
                            st1 = stages[-1]
                            fp_finish(strips[st1["s"]], st1["r3"], st1["s"],
                                      st1["rows"])
                            st1["done"] = True
                    stages.append({"r3": r3, "s": s, "ssq": ln_presum(sz, r3)})
                last = stages[-1]
                last["rows"] = ln_redrows(strips[last["s"]], last["ssq"])
                fp_finish(strips[last["s"]], last["r3"], last["s"],
                          last["rows"])

    nc.compile()
    return nc


# ---------------- host side ----------------

_BUILT = {}


def _get_nc(key):
    if key not in _BUILT:
        _BUILT[key] = build(*key)
    return _BUILT[key]


def _packv(v, ktiles):
    return np.ascontiguousarray(np.asarray(v, np.float32).reshape(ktiles, P).T)


def prep_inputs(inputs, bs=BS, n_cores=N_CORES):
    f32 = np.float32
    g = lambda n: np.asarray(inputs[n], f32)
    g_img, b_img = g("ln_img_g"), g("ln_img_b")
    g_txt, b_txt = g("ln_text_g"), g("ln_text_b")
    c_img = b_img + g("fi_b2")  # constant the img trunk omits
    c_txt = b_txt + g("ft_b2")
    common = {}
    # --- attention (bf16): Wc = Wo@Wv; ti's rows absorb g_img, bias absorbs
    # the img trunk's missing constant c_img.
    wc_it = g("it_Wo") @ g("it_Wv")
    bc_it = g("it_Wo") @ g("it_bv") + g("it_bo")
    common["watt_it"] = np.ascontiguousarray(wc_it.T).astype(BF16)
    common["batt_it"] = _packv(bc_it, KE)
    wc_ti = g("ti_Wo") @ g("ti_Wv")
    bc_ti = g("ti_Wo") @ g("ti_bv") + g("ti_bo") + wc_ti @ c_img
    common["watt_ti"] = np.ascontiguousarray(wc_ti.T * g_img[:, None]).astype(BF16)
    common["batt_ti"] = _packv(bc_ti, KE)
    # --- FFN (fp8): W1 rows absorb g_ln (input is t, not x); W2 columns are
    # divided by g_ln (trunk carries t + ffn/g); biases b1 absorb b_ln@W1.T.
    for p, gl, bl in (("fi", g_img, b_img), ("ft", g_txt, b_txt)):
        w1 = g(f"{p}_W1")  # [F, E]
        w2 = g(f"{p}_W2")  # [E, F]
        common[f"w1_{p}"] = np.ascontiguousarray(
            w1.T * (gl[:, None] * SW)).astype(F8)
        common[f"w2_{p}"] = np.ascontiguousarray(
            w2.T * (SW / gl[None, :])).astype(F8)
        common[f"b1_{p}"] = _packv(g(f"{p}_b1") + w1 @ bl, KF)
    # --- fused projection (bf16): rows absorb [g_img; g_txt]; bias absorbs
    # the trunks' missing constants.
    fpw = g("fp_W")  # [E, 2E]
    g_cat = np.concatenate([g_img, g_txt])
    c_cat = np.concatenate([c_img, c_txt])
    common["wfp"] = np.ascontiguousarray(fpw.T * g_cat[:, None]).astype(BF16)
    common["bfp"] = _packv(g("fp_b") + fpw @ c_cat, KE)
    common["g_fp"] = _packv(g("fp_ln_g"), KE)
    common["b_fp_ln"] = _packv(g("fp_ln_b"), KE)

    imgT = g("image_embed").T.astype(BF16)
    txtT = g("text_embed").T.astype(BF16)
    in_maps = []
    for c in range(n_cores):
        sl = slice(c * bs, (c + 1) * bs)
        m = dict(common)
        m["imageT"] = np.ascontiguousarray(imgT[:, sl])
        m["textT"] = np.ascontiguousarray(txtT[:, sl])
        in_maps.append(m)
    return in_maps


CFG = (BS, STRIPS, True, N_CORES)


def kernel(**inputs):
    nc = _get_nc(CFG)
    in_maps = prep_inputs(inputs)
    res = run_bass_kernel_spmd(nc, in_maps, core_ids=list(range(N_CORES)))
    out = np.concatenate(
        [res.results[c]["outT"] for c in range(N_CORES)], axis=1
    )  # [E, B]
    return np.ascontiguousarray(out.T).astype(np.float32)
